# revision 24
# baseline (speedup 1.0000x reference)
"""Multi-head attention (b=2, p=16, n=512, d=512, h=8, dh=64) on 8 TRN2 cores.

Data-parallel over the 32 (b,p) sequences: 4 sequences per core, no
collectives.  Per-core dataflow (everything "T" = feature-on-partition):

  xT  (d,n)  --W_qkv stationary-->  qT,kT (e,n)   [e-tile = 2 heads]
  xT chunks stationary, W_v moving ->  v natural (n,e)  -> vaug (j,h,65)
  dotsT[j,i] = kT_h.T-slice @ qT_h   (K=64, heads A/B at rows 0:64/64:128)
  expT = exp(scale * dotsT)          (ScalarE, PSUM->SBUF, bf16 out)
  oT[dh,i] (+ sums row 64) = vaug_h.T @ expT_h   (M=65, ones column -> sums)
  softmax denom (per head pair, no DRAM bounce):
    sums rows -> sflat [1,1024] (ScalarE) -> SBUF scatter DMA [128,8] ->
    reciprocal_approx_fast (DVE) -> bf16 cast -> SBUF gather DMA [2,512] ->
    R = P2.T @ rec (PE broadcast) -> oT *= R (DVE, in place)
  yT = W_out.T @ oT + b  per 128-col chunk, each chunk DMAed out as done.

Scheduling: one software pipeline.  QKV e-tiles are loaded just-in-time
(q0/k0 of a sequence first, then v tiles, then the rest), the previous
sequence's output projection and the next sequence's QKV are interleaved
into the attention pairs so the TensorEngine never starves.  Input DMAs
are chunked (per e-tile / per dt) so the first matmul starts ~1.5us in.
"""

import os
import sys

import numpy as np

for _p in ("/opt/trn_rl_repo", "/root/.axon_site/_ro/trn_rl_repo"):
    if os.path.isdir(_p) and _p not in sys.path:
        sys.path.insert(0, _p)

import concourse.bass as bass  # noqa: E402
import concourse.mybir as mybir  # noqa: E402
from concourse import bacc  # noqa: E402
from concourse.tile import TileContext  # noqa: E402

F32 = mybir.dt.float32
BF16 = mybir.dt.bfloat16
F32R = mybir.dt.float32r

N_CORES = 8
SEQ_PER_CORE = 4  # (b*p)=32 sequences / 8 cores
N = 512  # tokens per sequence
D = 512  # model dim
HEADS = 8
DH = 64
SCALE = DH**-0.5
NT = N // 128  # 4 token tiles
DT = D // 128  # 4 dim tiles

EXP_F = mybir.ActivationFunctionType.Exp
COPY_F = mybir.ActivationFunctionType.Copy
MULT = mybir.AluOpType.mult


def build_nc():
    """Build the per-core SPMD Bass program (same program on all 8 cores)."""
    nc = bacc.Bacc("TRN2", target_bir_lowering=False)

    xT = nc.declare_dram_parameter(
        "xT", [SEQ_PER_CORE, DT, 128, N], BF16, isOutput=False
    )
    # q e-tiles 0..3 then k e-tiles 0..3, each prepacked [128, DT, 128]
    wqk = nc.declare_dram_parameter("wqk", [8, 128, DT, 128], BF16, isOutput=False)
    wv = nc.declare_dram_parameter("wv", [128, DT, D], BF16, isOutput=False)
    wout = nc.declare_dram_parameter("wout", [128, DT, D], BF16, isOutput=False)
    bout = nc.declare_dram_parameter("bout", [D], F32, isOutput=False)
    p2d = nc.declare_dram_parameter("p2d", [1, 256], BF16, isOutput=False)
    out = nc.declare_dram_parameter(
        "out", [SEQ_PER_CORE, DT, 128, N], F32, isOutput=True
    )

    with TileContext(nc) as tc:
        with (
            tc.tile_pool(name="consts", bufs=1) as cpool,
            tc.tile_pool(name="xin", bufs=2) as xpool,
            tc.tile_pool(name="qk", bufs=2) as qkpool,
            tc.tile_pool(name="vaug", bufs=2) as vpool,
            tc.tile_pool(name="expt", bufs=3) as epool,
            tc.tile_pool(name="ot", bufs=2) as opool,
            tc.tile_pool(name="small", bufs=2) as spool,
            tc.tile_pool(name="yout", bufs=4) as ypool,
            tc.tile_pool(name="psq", bufs=2, space="PSUM") as psq,
            tc.tile_pool(name="psd", bufs=1, space="PSUM") as psd,
            tc.tile_pool(name="pso", bufs=1, space="PSUM") as pso,
        ):
            # ---- constants (DMA issue order matters: first-needed first;
            # all weight tensors host-prepacked so each DMA is contiguous) --
            wqk_sb = cpool.tile([128, 8, DT, 128], BF16, tag="wqk")

            seq_x = {}

            def x_alloc(s, interleave_et0=False):
                xts = [
                    xpool.tile([128, N], BF16, tag=f"x{dt}", name=f"x{s}_{dt}")
                    for dt in range(DT)
                ]
                for dt in range(DT):
                    # q0/k0 weight chunks interleaved per dt with x so the
                    # first matmul (q0, dt 0) starts after ~64KB has landed
                    if interleave_et0:
                        nc.sync.dma_start(wqk_sb[:, 0, dt], wqk[0, :, dt])
                    nc.sync.dma_start(xts[dt][:], xT[s, dt])
                seq_x[s] = xts

            x_alloc(0, interleave_et0=True)
            for dt in range(DT):
                nc.sync.dma_start(wqk_sb[:, 4, dt], wqk[4, :, dt])

            p2 = cpool.tile([1, 256], BF16, tag="p2")
            nc.sync.dma_start(p2[:], p2d[:])
            wv_sb = cpool.tile([128, DT, D], BF16, tag="wv")
            nc.sync.dma_start(wv_sb[:], wv[:])
            for et in (1, 5, 2, 6, 3, 7):
                nc.sync.dma_start(wqk_sb[:, et], wqk[et])
            wo_sb = cpool.tile([128, DT, D], BF16, tag="wo")
            nc.sync.dma_start(wo_sb[:], wout[:])
            b_sb = cpool.tile([128, DT], F32, tag="b")
            nc.sync.dma_start(b_sb[:], bout.rearrange("(t p) -> p t", p=128))

            seq_tiles = {}

            def qkv_alloc(s):
                q_sb = qkpool.tile([128, DT, N], BF16, tag="q", name=f"q{s}")
                k_sb = qkpool.tile([128, DT, N], BF16, tag="k", name=f"k{s}")
                vaug = vpool.tile(
                    [128, NT, HEADS, DH + 1], BF16, tag="v", name=f"v{s}"
                )
                nc.vector.memset(vaug[:, :, :, DH : DH + 1], 1.0)
                seq_tiles[s] = (q_sb, k_sb, vaug)

            def qkv_etile(s, kind, idx):
                """One QKV output tile: 4 accumulating matmuls + evacuation.
                kind 'q'/'k': e-tile idx (head pair idx); 'v': n-tile idx."""
                q_sb, k_sb, vaug = seq_tiles[s]
                xts = seq_x[s]
                ps = psq.tile([128, 512], F32, tag="ps", name=f"ps_{kind}{s}_{idx}")
                if kind in ("q", "k"):
                    et = idx if kind == "q" else 4 + idx
                    for dt in range(DT):
                        nc.tensor.matmul(
                            ps[:],
                            lhsT=wqk_sb[:, et, dt, :],
                            rhs=xts[dt][:],
                            start=(dt == 0),
                            stop=(dt == DT - 1),
                        )
                    dest = q_sb if kind == "q" else k_sb
                    nc.vector.tensor_copy(dest[:, idx, :], ps[:])
                else:
                    nt = idx
                    for dt in range(DT):
                        nc.tensor.matmul(
                            ps[:],
                            lhsT=xts[dt][:, nt * 128 : (nt + 1) * 128],
                            rhs=wv_sb[:, dt, :],
                            start=(dt == 0),
                            stop=(dt == DT - 1),
                        )
                    nc.vector.tensor_copy(
                        vaug[:, nt, :, 0:DH],
                        ps.rearrange("p (h d) -> p h d", h=HEADS),
                    )

            rec2s = {}

            def norm_collect(s, t, oAB):
                """Sums rows -> 1/sums -> rec2 (bf16 [1,1024]), per pair.
                No DMA anywhere in the chain; the off-critical copies ride
                the idle Pool engine, except for the final pair whose chain
                is exposed at the kernel tail."""
                last = s == SEQ_PER_CORE - 1 and t == 3
                recf = spool.tile([1, 1024], F32, tag="recf", name=f"rf{s}_{t}")
                nc.scalar.activation(recf[0:1, :], oAB[64:65, :], COPY_F)
                nc.vector.reciprocal_approx_fast(recf[:], recf[:])
                rec2 = spool.tile([1, 1024], BF16, tag="rec2", name=f"r2_{s}_{t}")
                with nc.allow_low_precision(reason="softmax recip bf16"):
                    (nc.vector if last else nc.gpsimd).tensor_copy(
                        rec2[:], recf[:]
                    )
                rec2s[(s, t)] = rec2

            def norm_pe(s, t):
                """R broadcast matmul + oT *= R for pair t of sequence s."""
                oT = seq_o[s]
                Rp = psq.tile([128, 512], F32, tag="ps", name=f"Rp{s}_{t}")
                rec2 = rec2s[(s, t)]
                nc.tensor.matmul(
                    Rp[:],
                    lhsT=p2[0:1, 0:128],
                    rhs=rec2[0:1, 0:512],
                    start=True,
                    stop=False,
                )
                nc.tensor.matmul(
                    Rp[:],
                    lhsT=p2[0:1, 128:256],
                    rhs=rec2[0:1, 512:1024],
                    start=False,
                    stop=True,
                )
                nc.vector.tensor_tensor(oT[:, t, :], oT[:, t, :], Rp[:], MULT)

            seq_o = {}

            def proj_dt(s, dt):
                """One 128-col chunk of the output projection + store."""
                oT = seq_o[s]
                ps = psq.tile([128, 512], F32, tag="ps", name=f"pj{s}_{dt}")
                for et in range(DT):
                    nc.tensor.matmul(
                        ps[:],
                        lhsT=wo_sb[:, et, dt * 128 : (dt + 1) * 128],
                        rhs=oT[:, et, :],
                        start=(et == 0),
                        stop=(et == DT - 1),
                    )
                yt = ypool.tile([128, 512], F32, tag="y", name=f"yt{s}_{dt}")
                nc.vector.tensor_scalar_add(yt[:], ps[:], b_sb[:, dt : dt + 1])
                nc.sync.dma_start(out[s, dt], yt[:])

            # ---- prologue: seq 0 q0/k0/v0/v1 -----------------------------
            qkv_alloc(0)
            qkv_etile(0, "q", 0)
            qkv_etile(0, "k", 0)
            qkv_etile(0, "v", 0)
            qkv_etile(0, "v", 1)

            for s in range(SEQ_PER_CORE):
                q_sb, k_sb, vaug = seq_tiles[s]
                oT = opool.tile([128, DT, N], BF16, tag="o", name=f"oT{s}")
                seq_o[s] = oT

                # filler units consumed inside the pair loop below.  Late
                # QKV e-tiles of sequence s ride in s's own stream (q2/k2
                # before pair 2, q3/k3 before pair 3); the early e-tiles of
                # s+1 (q0/k0 + all v) complete during s so s+1's pair 0 can
                # start immediately.
                fill = []

                def et_units(s_, pairs):
                    return [
                        (lambda k__=k, i__=i, s__=s_: qkv_etile(s__, k__, i__))
                        for (k, i) in pairs
                    ]

                if s == 0:
                    fill += et_units(0, [("v", 2), ("v", 3), ("q", 1), ("k", 1)])
                else:
                    fill += et_units(s, [("q", 2), ("k", 2)])
                    fill.append(lambda s_=s - 1: norm_pe(s_, 3))
                    fill += [
                        (lambda s_=s - 1, d_=d: proj_dt(s_, d_)) for d in range(DT)
                    ]
                if s == 0:
                    fill += et_units(0, [("q", 2), ("k", 2), ("q", 3), ("k", 3)])
                else:
                    fill += et_units(s, [("q", 3), ("k", 3)])
                if s + 1 < SEQ_PER_CORE:
                    qkv_alloc(s + 1)
                    fill += et_units(
                        s + 1,
                        [("q", 0), ("k", 0),
                         ("v", 0), ("v", 1), ("v", 2), ("v", 3),
                         ("q", 1), ("k", 1)],
                    )
                fi = 0

                def filler(k):
                    nonlocal fi
                    for _ in range(k):
                        if fi < len(fill):
                            fill[fi]()
                            fi += 1

                for t in range(4):  # head pair (2t, 2t+1)
                    if t == 1 and s + 1 < SEQ_PER_CORE:
                        x_alloc(s + 1)
                    expA = epool.tile([128, NT, N], BF16, tag="expA")
                    expB = epool.tile([128, NT, N], BF16, tag="expB")
                    oAB = pso.tile([128, 1024], F32, tag="oAB")

                    def dots(jh):
                        dA = psd.tile([128, 1024], F32, tag="dA")
                        dB = psd.tile([128, 1024], F32, tag="dB")
                        for jj in range(2):
                            jt = 2 * jh + jj
                            nc.tensor.matmul(
                                dA[:, jj * 512 : (jj + 1) * 512],
                                lhsT=k_sb[0:64, t, jt * 128 : (jt + 1) * 128],
                                rhs=q_sb[0:64, t, :],
                                start=True,
                                stop=True,
                            )
                            nc.tensor.matmul(
                                dB[:, jj * 512 : (jj + 1) * 512],
                                lhsT=k_sb[64:128, t, jt * 128 : (jt + 1) * 128],
                                rhs=q_sb[64:128, t, :],
                                start=True,
                                stop=True,
                            )
                        nc.scalar.activation(
                            expA[:, 2 * jh : 2 * jh + 2, :],
                            dA.rearrange("p (a n) -> p a n", a=2),
                            EXP_F,
                            scale=SCALE,
                        )
                        nc.scalar.activation(
                            expB[:, 2 * jh : 2 * jh + 2, :],
                            dB.rearrange("p (a n) -> p a n", a=2),
                            EXP_F,
                            scale=SCALE,
                        )

                    def attnv(jh):
                        for jj in range(2):
                            jt = 2 * jh + jj
                            nc.tensor.matmul(
                                oAB[0:65, 0:512],
                                lhsT=vaug[:, jt, 2 * t, :],
                                rhs=expA[:, jt, :],
                                start=(jt == 0),
                                stop=(jt == NT - 1),
                            )
                            nc.tensor.matmul(
                                oAB[0:65, 512:1024],
                                lhsT=vaug[:, jt, 2 * t + 1, :],
                                rhs=expB[:, jt, :],
                                start=(jt == 0),
                                stop=(jt == NT - 1),
                            )

                    dots(0)
                    filler(1)
                    dots(1)
                    filler(1)
                    if t > 0:
                        norm_pe(s, t - 1)
                    filler(1)
                    attnv(0)
                    filler(1)
                    attnv(1)

                    # evacuate unnormalized oT (f32 PSUM -> bf16 SBUF)
                    nc.vector.tensor_copy(oT[0:64, t, :], oAB[0:64, 0:512])
                    nc.vector.tensor_copy(oT[64:128, t, :], oAB[0:64, 512:1024])
                    norm_collect(s, t, oAB)

                    if s == SEQ_PER_CORE - 1 and t == 3:
                        # Final projection, split: e-tiles 0..2 accumulate
                        # into the PSUM banks the last exps just freed,
                        # running under pair 3's norm chain; only the et=3
                        # matmuls + bias + store remain after the last
                        # oT *= R.
                        pj01 = psd.tile([128, 1024], F32, tag="dA", name="pj01")
                        pj23 = psd.tile([128, 1024], F32, tag="dB", name="pj23")
                        for et in range(DT - 1):
                            for dt in range(DT):
                                pj = pj01 if dt < 2 else pj23
                                col = (dt % 2) * 512
                                nc.tensor.matmul(
                                    pj[:, col : col + 512],
                                    lhsT=wo_sb[:, et, dt * 128 : (dt + 1) * 128],
                                    rhs=oT[:, et, :],
                                    start=(et == 0),
                                    stop=False,
                                )

                # drain leftover fillers, then close out this sequence
                filler(len(fill))
                if s == SEQ_PER_CORE - 1:
                    norm_pe(s, 3)
                    for dt in range(DT):
                        pj = pj01 if dt < 2 else pj23
                        col = (dt % 2) * 512
                        nc.tensor.matmul(
                            pj[:, col : col + 512],
                            lhsT=wo_sb[:, DT - 1, dt * 128 : (dt + 1) * 128],
                            rhs=oT[:, DT - 1, :],
                            start=False,
                            stop=True,
                        )
                    for dt in range(DT):
                        pj = pj01 if dt < 2 else pj23
                        col = (dt % 2) * 512
                        yt = ypool.tile([128, 512], F32, tag="y", name=f"ytL{dt}")
                        nc.vector.tensor_scalar_add(
                            yt[:], pj[:, col : col + 512], b_sb[:, dt : dt + 1]
                        )
                        nc.sync.dma_start(out[s, dt], yt[:])

    nc.compile()
    return nc


def make_in_maps(x, W_qkv, W_out, b_out):
    """Shard + lay out full inputs into the 8 per-core input maps."""
    import ml_dtypes

    b, p, n, d = x.shape
    xs = np.ascontiguousarray(x, dtype=np.float32).reshape(b * p, n, d)
    Wb = (
        np.ascontiguousarray(W_qkv, dtype=np.float32)
        .reshape(DT, 128, 3, 4, 128)
        .astype(ml_dtypes.bfloat16)
    )
    # [8, 128, DT, 128] (partition-major): q e-tiles 0..3 then k e-tiles 0..3
    wqk = np.ascontiguousarray(
        np.concatenate(
            [Wb[:, :, 0].transpose(2, 1, 0, 3), Wb[:, :, 1].transpose(2, 1, 0, 3)]
        )
    )
    # [128, DT, D] (partition-major)
    wv = np.ascontiguousarray(Wb[:, :, 2].reshape(DT, 128, D).transpose(1, 0, 2))
    wout = np.ascontiguousarray(
        np.ascontiguousarray(W_out, dtype=np.float32)
        .reshape(DT, 128, D)
        .astype(ml_dtypes.bfloat16)
        .transpose(1, 0, 2)
    )
    bo = np.ascontiguousarray(b_out, dtype=np.float32)

    in_maps = []
    for c in range(N_CORES):
        seqs = xs[c * SEQ_PER_CORE : (c + 1) * SEQ_PER_CORE]  # (4, n, d)
        xT = (
            np.ascontiguousarray(seqs.transpose(0, 2, 1))
            .reshape(SEQ_PER_CORE, DT, 128, N)
            .astype(ml_dtypes.bfloat16)
        )
        p2 = np.zeros((1, 256), dtype=ml_dtypes.bfloat16)
        p2[0, 0:64] = 1.0
        p2[0, 128 + 64 : 256] = 1.0
        in_maps.append(
            {"xT": xT, "wqk": wqk, "wv": wv, "wout": wout, "bout": bo, "p2d": p2}
        )
    return in_maps


def assemble_output(results, b, p, n, d):
    """Gather per-core yT outputs back into the full (b,p,n,d) array."""
    y = np.empty((b * p, n, d), dtype=np.float32)
    for c in range(N_CORES):
        yT = np.asarray(results[c]["out"]).reshape(SEQ_PER_CORE, D, N)
        y[c * SEQ_PER_CORE : (c + 1) * SEQ_PER_CORE] = yT.transpose(0, 2, 1)
    return y.reshape(b, p, n, d)


_NC_CACHE = None


def _get_nc():
    global _NC_CACHE
    if _NC_CACHE is None:
        _NC_CACHE = build_nc()
    return _NC_CACHE


def run(inputs, trace=False, **spmd_kwargs):
    """Run on the 8 NeuronCores; returns (full_output, BassKernelResults)."""
    from concourse.bass_utils import run_bass_kernel_spmd

    x = np.asarray(inputs["x"])
    b, p, n, d = x.shape
    nc = _get_nc()
    in_maps = make_in_maps(x, inputs["W_qkv"], inputs["W_out"], inputs["b_out"])
    res = run_bass_kernel_spmd(
        nc, in_maps, core_ids=list(range(N_CORES)), trace=trace, **spmd_kwargs
    )
    return assemble_output(res.results, b, p, n, d), res


def kernel(x, W_qkv, W_out, b_out):
    out, _ = run({"x": x, "W_qkv": W_qkv, "W_out": W_out, "b_out": b_out})
    return out.astype(np.float32)


# revision 25
# speedup vs baseline: 1.2779x; 1.2779x over previous
"""Multi-head attention (b=2, p=16, n=512, d=512, h=8, dh=64) on 8 TRN2 cores.

Data-parallel over the 32 (b,p) sequences: 4 sequences per core, no
collectives.  Per-core dataflow (everything "T" = feature-on-partition):

  xT  (d,n)  --W_qkv stationary-->  qT,kT (e,n)   [e-tile = 2 heads]
  xT chunks stationary, W_v moving ->  v natural (n,e)  -> vaug (j,h,65)
  dotsT[j,i] = kT_h.T-slice @ qT_h   (K=64, heads A/B at rows 0:64/64:128)
  expT = exp(scale * dotsT)          (ScalarE, PSUM->SBUF, bf16 out)
  oT[dh,i] (+ sums row 64) = vaug_h.T @ expT_h   (M=65, ones column -> sums)
  softmax denom (per head pair, no DRAM bounce):
    sums rows -> sflat [1,1024] (ScalarE) -> SBUF scatter DMA [128,8] ->
    reciprocal_approx_fast (DVE) -> bf16 cast -> SBUF gather DMA [2,512] ->
    R = P2.T @ rec (PE broadcast) -> oT *= R (DVE, in place)
  yT = W_out.T @ oT + b  per 128-col chunk, each chunk DMAed out as done.

Scheduling: one software pipeline.  QKV e-tiles are loaded just-in-time
(q0/k0 of a sequence first, then v tiles, then the rest), the previous
sequence's output projection and the next sequence's QKV are interleaved
into the attention pairs so the TensorEngine never starves.  Input DMAs
are chunked (per e-tile / per dt) so the first matmul starts ~1.5us in.
"""

import os
import sys

import numpy as np

for _p in ("/opt/trn_rl_repo", "/root/.axon_site/_ro/trn_rl_repo"):
    if os.path.isdir(_p) and _p not in sys.path:
        sys.path.insert(0, _p)

import concourse.bass as bass  # noqa: E402
import concourse.mybir as mybir  # noqa: E402
from concourse import bacc  # noqa: E402
from concourse.tile import TileContext  # noqa: E402

F32 = mybir.dt.float32
BF16 = mybir.dt.bfloat16
F32R = mybir.dt.float32r

N_CORES = 8
SEQ_PER_CORE = 4  # (b*p)=32 sequences / 8 cores
N = 512  # tokens per sequence
D = 512  # model dim
HEADS = 8
DH = 64
SCALE = DH**-0.5
NT = N // 128  # 4 token tiles
DT = D // 128  # 4 dim tiles

EXP_F = mybir.ActivationFunctionType.Exp
COPY_F = mybir.ActivationFunctionType.Copy
MULT = mybir.AluOpType.mult


def build_nc():
    """Build the per-core SPMD Bass program (same program on all 8 cores)."""
    nc = bacc.Bacc("TRN2", target_bir_lowering=False)

    xT = nc.declare_dram_parameter(
        "xT", [SEQ_PER_CORE, DT, 128, N], BF16, isOutput=False
    )
    # q e-tiles 0..3 then k e-tiles 0..3, each prepacked [128, DT, 128]
    wqk = nc.declare_dram_parameter("wqk", [8, 128, DT, 128], BF16, isOutput=False)
    wv = nc.declare_dram_parameter("wv", [128, DT, D], BF16, isOutput=False)
    wout = nc.declare_dram_parameter("wout", [128, DT, D], BF16, isOutput=False)
    bout = nc.declare_dram_parameter("bout", [D], F32, isOutput=False)
    p2d = nc.declare_dram_parameter("p2d", [1, 256], BF16, isOutput=False)
    out = nc.declare_dram_parameter(
        "out", [SEQ_PER_CORE, DT, 128, N], F32, isOutput=True
    )

    with TileContext(nc) as tc:
        with (
            tc.tile_pool(name="consts", bufs=1) as cpool,
            tc.tile_pool(name="xin", bufs=2) as xpool,
            tc.tile_pool(name="qk", bufs=2) as qkpool,
            tc.tile_pool(name="vaug", bufs=2) as vpool,
            tc.tile_pool(name="expt", bufs=3) as epool,
            tc.tile_pool(name="ot", bufs=2) as opool,
            tc.tile_pool(name="small", bufs=2) as spool,
            tc.tile_pool(name="yout", bufs=4) as ypool,
            tc.tile_pool(name="psq", bufs=2, space="PSUM") as psq,
            tc.tile_pool(name="psd", bufs=1, space="PSUM") as psd,
            tc.tile_pool(name="pso", bufs=1, space="PSUM") as pso,
        ):
            # ---- constants (DMA issue order matters: first-needed first;
            # all weight tensors host-prepacked so each DMA is contiguous) --
            wqk_sb = cpool.tile([128, 8, DT, 128], BF16, tag="wqk")

            seq_x = {}

            def x_alloc(s, interleave_et0=False):
                xts = [
                    xpool.tile([128, N], BF16, tag=f"x{dt}", name=f"x{s}_{dt}")
                    for dt in range(DT)
                ]
                for dt in range(DT):
                    # q0/k0 weight chunks interleaved per dt with x so the
                    # first matmul (q0, dt 0) starts after ~64KB has landed
                    if interleave_et0:
                        nc.sync.dma_start(wqk_sb[:, 0, dt], wqk[0, :, dt])
                    nc.sync.dma_start(xts[dt][:], xT[s, dt])
                seq_x[s] = xts

            x_alloc(0, interleave_et0=True)
            for dt in range(DT):
                nc.sync.dma_start(wqk_sb[:, 4, dt], wqk[4, :, dt])

            p2 = cpool.tile([1, 256], BF16, tag="p2")
            nc.sync.dma_start(p2[:], p2d[:])
            wv_sb = cpool.tile([128, DT, D], BF16, tag="wv")
            nc.sync.dma_start(wv_sb[:], wv[:])
            for et in (1, 5, 2, 6, 3, 7):
                nc.sync.dma_start(wqk_sb[:, et], wqk[et])
            wo_sb = cpool.tile([128, DT, D], BF16, tag="wo")
            nc.sync.dma_start(wo_sb[:], wout[:])
            b_sb = cpool.tile([128, DT], F32, tag="b")
            nc.sync.dma_start(b_sb[:], bout.rearrange("(t p) -> p t", p=128))

            seq_tiles = {}

            def qkv_alloc(s):
                q_sb = qkpool.tile([128, DT, N], BF16, tag="q", name=f"q{s}")
                k_sb = qkpool.tile([128, DT, N], BF16, tag="k", name=f"k{s}")
                vaug = vpool.tile(
                    [128, NT, HEADS, DH + 1], BF16, tag="v", name=f"v{s}"
                )
                nc.vector.memset(vaug[:, :, :, DH : DH + 1], 1.0)
                seq_tiles[s] = (q_sb, k_sb, vaug)

            def qkv_etile(s, kind, idx):
                """One QKV output tile: 4 accumulating matmuls + evacuation.
                kind 'q'/'k': e-tile idx (head pair idx); 'v': n-tile idx."""
                q_sb, k_sb, vaug = seq_tiles[s]
                xts = seq_x[s]
                ps = psq.tile([128, 512], F32, tag="ps", name=f"ps_{kind}{s}_{idx}")
                if kind in ("q", "k"):
                    et = idx if kind == "q" else 4 + idx
                    for dt in range(DT):
                        nc.tensor.matmul(
                            ps[:],
                            lhsT=wqk_sb[:, et, dt, :],
                            rhs=xts[dt][:],
                            start=(dt == 0),
                            stop=(dt == DT - 1),
                        )
                    dest = q_sb if kind == "q" else k_sb
                    nc.vector.tensor_copy(dest[:, idx, :], ps[:])
                else:
                    nt = idx
                    for dt in range(DT):
                        nc.tensor.matmul(
                            ps[:],
                            lhsT=xts[dt][:, nt * 128 : (nt + 1) * 128],
                            rhs=wv_sb[:, dt, :],
                            start=(dt == 0),
                            stop=(dt == DT - 1),
                        )
                    nc.vector.tensor_copy(
                        vaug[:, nt, :, 0:DH],
                        ps.rearrange("p (h d) -> p h d", h=HEADS),
                    )

            rec2s = {}

            def norm_collect(s, t, oAB):
                """Sums rows -> 1/sums -> rec2 (bf16 [1,1024]), per pair.
                No DMA anywhere in the chain; the off-critical copies ride
                the idle Pool engine, except for the final pair whose chain
                is exposed at the kernel tail."""
                recf = spool.tile([1, 1024], F32, tag="recf", name=f"rf{s}_{t}")
                nc.scalar.activation(recf[0:1, :], oAB[64:65, :], COPY_F)
                nc.vector.reciprocal_approx_fast(recf[:], recf[:])
                rec2 = spool.tile([1, 1024], BF16, tag="rec2", name=f"r2_{s}_{t}")
                with nc.allow_low_precision(reason="softmax recip bf16"):
                    nc.vector.tensor_copy(rec2[:], recf[:])
                rec2s[(s, t)] = rec2

            def norm_pe(s, t):
                """R broadcast matmul + oT *= R for pair t of sequence s."""
                oT = seq_o[s]
                Rp = psq.tile([128, 512], F32, tag="ps", name=f"Rp{s}_{t}")
                rec2 = rec2s[(s, t)]
                nc.tensor.matmul(
                    Rp[:],
                    lhsT=p2[0:1, 0:128],
                    rhs=rec2[0:1, 0:512],
                    start=True,
                    stop=False,
                )
                nc.tensor.matmul(
                    Rp[:],
                    lhsT=p2[0:1, 128:256],
                    rhs=rec2[0:1, 512:1024],
                    start=False,
                    stop=True,
                )
                nc.vector.tensor_tensor(oT[:, t, :], oT[:, t, :], Rp[:], MULT)

            seq_o = {}

            def proj_dt(s, dt):
                """One 128-col chunk of the output projection + store."""
                oT = seq_o[s]
                ps = psq.tile([128, 512], F32, tag="ps", name=f"pj{s}_{dt}")
                for et in range(DT):
                    nc.tensor.matmul(
                        ps[:],
                        lhsT=wo_sb[:, et, dt * 128 : (dt + 1) * 128],
                        rhs=oT[:, et, :],
                        start=(et == 0),
                        stop=(et == DT - 1),
                    )
                yt = ypool.tile([128, 512], F32, tag="y", name=f"yt{s}_{dt}")
                nc.vector.tensor_scalar_add(yt[:], ps[:], b_sb[:, dt : dt + 1])
                nc.sync.dma_start(out[s, dt], yt[:])

            # ---- prologue: seq 0 q0/k0/v0/v1 -----------------------------
            qkv_alloc(0)
            qkv_etile(0, "q", 0)
            qkv_etile(0, "k", 0)
            qkv_etile(0, "v", 0)
            qkv_etile(0, "v", 1)

            for s in range(SEQ_PER_CORE):
                q_sb, k_sb, vaug = seq_tiles[s]
                oT = opool.tile([128, DT, N], BF16, tag="o", name=f"oT{s}")
                seq_o[s] = oT

                # filler units consumed inside the pair loop below.  Late
                # QKV e-tiles of sequence s ride in s's own stream (q2/k2
                # before pair 2, q3/k3 before pair 3); the early e-tiles of
                # s+1 (q0/k0 + all v) complete during s so s+1's pair 0 can
                # start immediately.
                fill = []

                def et_units(s_, pairs):
                    return [
                        (lambda k__=k, i__=i, s__=s_: qkv_etile(s__, k__, i__))
                        for (k, i) in pairs
                    ]

                if s == 0:
                    fill += et_units(0, [("v", 2), ("v", 3), ("q", 1), ("k", 1)])
                else:
                    fill += et_units(s, [("q", 2), ("k", 2)])
                    fill.append(lambda s_=s - 1: norm_pe(s_, 3))
                    fill += [
                        (lambda s_=s - 1, d_=d: proj_dt(s_, d_)) for d in range(DT)
                    ]
                if s == 0:
                    fill += et_units(0, [("q", 2), ("k", 2), ("q", 3), ("k", 3)])
                else:
                    fill += et_units(s, [("q", 3), ("k", 3)])
                if s + 1 < SEQ_PER_CORE:
                    qkv_alloc(s + 1)
                    fill += et_units(
                        s + 1,
                        [("q", 0), ("k", 0),
                         ("v", 0), ("v", 1), ("v", 2), ("v", 3),
                         ("q", 1), ("k", 1)],
                    )
                fi = 0

                def filler(k):
                    nonlocal fi
                    for _ in range(k):
                        if fi < len(fill):
                            fill[fi]()
                            fi += 1

                for t in range(4):  # head pair (2t, 2t+1)
                    if t == 1 and s + 1 < SEQ_PER_CORE:
                        x_alloc(s + 1)
                    expA = epool.tile([128, NT, N], BF16, tag="expA")
                    expB = epool.tile([128, NT, N], BF16, tag="expB")
                    oAB = pso.tile([128, 1024], F32, tag="oAB")

                    def dots(jh):
                        dA = psd.tile([128, 1024], F32, tag="dA")
                        dB = psd.tile([128, 1024], F32, tag="dB")
                        for jj in range(2):
                            jt = 2 * jh + jj
                            nc.tensor.matmul(
                                dA[:, jj * 512 : (jj + 1) * 512],
                                lhsT=k_sb[0:64, t, jt * 128 : (jt + 1) * 128],
                                rhs=q_sb[0:64, t, :],
                                start=True,
                                stop=True,
                            )
                            nc.tensor.matmul(
                                dB[:, jj * 512 : (jj + 1) * 512],
                                lhsT=k_sb[64:128, t, jt * 128 : (jt + 1) * 128],
                                rhs=q_sb[64:128, t, :],
                                start=True,
                                stop=True,
                            )
                        nc.scalar.activation(
                            expA[:, 2 * jh : 2 * jh + 2, :],
                            dA.rearrange("p (a n) -> p a n", a=2),
                            EXP_F,
                            scale=SCALE,
                        )
                        nc.scalar.activation(
                            expB[:, 2 * jh : 2 * jh + 2, :],
                            dB.rearrange("p (a n) -> p a n", a=2),
                            EXP_F,
                            scale=SCALE,
                        )

                    def attnv(jh):
                        for jj in range(2):
                            jt = 2 * jh + jj
                            nc.tensor.matmul(
                                oAB[0:65, 0:512],
                                lhsT=vaug[:, jt, 2 * t, :],
                                rhs=expA[:, jt, :],
                                start=(jt == 0),
                                stop=(jt == NT - 1),
                            )
                            nc.tensor.matmul(
                                oAB[0:65, 512:1024],
                                lhsT=vaug[:, jt, 2 * t + 1, :],
                                rhs=expB[:, jt, :],
                                start=(jt == 0),
                                stop=(jt == NT - 1),
                            )

                    dots(0)
                    filler(1)
                    dots(1)
                    filler(1)
                    if t > 0:
                        norm_pe(s, t - 1)
                    filler(1)
                    attnv(0)
                    filler(1)
                    attnv(1)

                    # evacuate unnormalized oT (f32 PSUM -> bf16 SBUF)
                    nc.vector.tensor_copy(oT[0:64, t, :], oAB[0:64, 0:512])
                    nc.vector.tensor_copy(oT[64:128, t, :], oAB[0:64, 512:1024])
                    norm_collect(s, t, oAB)

                    if s == SEQ_PER_CORE - 1 and t == 3:
                        # Final projection, split: e-tiles 0..2 accumulate
                        # into the PSUM banks the last exps just freed,
                        # running under pair 3's norm chain; only the et=3
                        # matmuls + bias + store remain after the last
                        # oT *= R.
                        pj01 = psd.tile([128, 1024], F32, tag="dA", name="pj01")
                        pj23 = psd.tile([128, 1024], F32, tag="dB", name="pj23")
                        for et in range(DT - 1):
                            for dt in range(DT):
                                pj = pj01 if dt < 2 else pj23
                                col = (dt % 2) * 512
                                nc.tensor.matmul(
                                    pj[:, col : col + 512],
                                    lhsT=wo_sb[:, et, dt * 128 : (dt + 1) * 128],
                                    rhs=oT[:, et, :],
                                    start=(et == 0),
                                    stop=False,
                                )

                # drain leftover fillers, then close out this sequence
                filler(len(fill))
                if s == SEQ_PER_CORE - 1:
                    norm_pe(s, 3)
                    for dt in range(DT):
                        pj = pj01 if dt < 2 else pj23
                        col = (dt % 2) * 512
                        nc.tensor.matmul(
                            pj[:, col : col + 512],
                            lhsT=wo_sb[:, DT - 1, dt * 128 : (dt + 1) * 128],
                            rhs=oT[:, DT - 1, :],
                            start=False,
                            stop=True,
                        )
                    for dt in range(DT):
                        pj = pj01 if dt < 2 else pj23
                        col = (dt % 2) * 512
                        yt = ypool.tile([128, 512], F32, tag="y", name=f"ytL{dt}")
                        nc.vector.tensor_scalar_add(
                            yt[:], pj[:, col : col + 512], b_sb[:, dt : dt + 1]
                        )
                        nc.sync.dma_start(out[s, dt], yt[:])

    nc.compile()
    return nc


def make_in_maps(x, W_qkv, W_out, b_out):
    """Shard + lay out full inputs into the 8 per-core input maps."""
    import ml_dtypes

    b, p, n, d = x.shape
    xs = np.ascontiguousarray(x, dtype=np.float32).reshape(b * p, n, d)
    Wb = (
        np.ascontiguousarray(W_qkv, dtype=np.float32)
        .reshape(DT, 128, 3, 4, 128)
        .astype(ml_dtypes.bfloat16)
    )
    # [8, 128, DT, 128] (partition-major): q e-tiles 0..3 then k e-tiles 0..3
    wqk = np.ascontiguousarray(
        np.concatenate(
            [Wb[:, :, 0].transpose(2, 1, 0, 3), Wb[:, :, 1].transpose(2, 1, 0, 3)]
        )
    )
    # [128, DT, D] (partition-major)
    wv = np.ascontiguousarray(Wb[:, :, 2].reshape(DT, 128, D).transpose(1, 0, 2))
    wout = np.ascontiguousarray(
        np.ascontiguousarray(W_out, dtype=np.float32)
        .reshape(DT, 128, D)
        .astype(ml_dtypes.bfloat16)
        .transpose(1, 0, 2)
    )
    bo = np.ascontiguousarray(b_out, dtype=np.float32)

    in_maps = []
    for c in range(N_CORES):
        seqs = xs[c * SEQ_PER_CORE : (c + 1) * SEQ_PER_CORE]  # (4, n, d)
        xT = (
            np.ascontiguousarray(seqs.transpose(0, 2, 1))
            .reshape(SEQ_PER_CORE, DT, 128, N)
            .astype(ml_dtypes.bfloat16)
        )
        p2 = np.zeros((1, 256), dtype=ml_dtypes.bfloat16)
        p2[0, 0:64] = 1.0
        p2[0, 128 + 64 : 256] = 1.0
        in_maps.append(
            {"xT": xT, "wqk": wqk, "wv": wv, "wout": wout, "bout": bo, "p2d": p2}
        )
    return in_maps


def assemble_output(results, b, p, n, d):
    """Gather per-core yT outputs back into the full (b,p,n,d) array."""
    y = np.empty((b * p, n, d), dtype=np.float32)
    for c in range(N_CORES):
        yT = np.asarray(results[c]["out"]).reshape(SEQ_PER_CORE, D, N)
        y[c * SEQ_PER_CORE : (c + 1) * SEQ_PER_CORE] = yT.transpose(0, 2, 1)
    return y.reshape(b, p, n, d)


_NC_CACHE = None


def _get_nc():
    global _NC_CACHE
    if _NC_CACHE is None:
        _NC_CACHE = build_nc()
    return _NC_CACHE


def run(inputs, trace=False, **spmd_kwargs):
    """Run on the 8 NeuronCores; returns (full_output, BassKernelResults)."""
    from concourse.bass_utils import run_bass_kernel_spmd

    x = np.asarray(inputs["x"])
    b, p, n, d = x.shape
    nc = _get_nc()
    in_maps = make_in_maps(x, inputs["W_qkv"], inputs["W_out"], inputs["b_out"])
    res = run_bass_kernel_spmd(
        nc, in_maps, core_ids=list(range(N_CORES)), trace=trace, **spmd_kwargs
    )
    return assemble_output(res.results, b, p, n, d), res


def kernel(x, W_qkv, W_out, b_out):
    out, _ = run({"x": x, "W_qkv": W_qkv, "W_out": W_out, "b_out": b_out})
    return out.astype(np.float32)


# revision 27
# speedup vs baseline: 1.3210x; 1.0337x over previous
"""Multi-head attention (b=2, p=16, n=512, d=512, h=8, dh=64) on 8 TRN2 cores.

Data-parallel over the 32 (b,p) sequences: 4 sequences per core, no
collectives.  Per-core dataflow (everything "T" = feature-on-partition):

  xT  (d,n)  --W_qkv stationary-->  qT,kT (e,n)   [e-tile = 2 heads]
  xT chunks stationary, W_v moving ->  v natural (n,e)  -> vaug (j,h,65)
  dotsT[j,i] = kT_h.T-slice @ qT_h   (K=64, heads A/B at rows 0:64/64:128)
  expT = exp(scale * dotsT)          (ScalarE, PSUM->SBUF, bf16 out)
  oT[dh,i] (+ sums row 64) = vaug_h.T @ expT_h   (M=65, ones column -> sums)
  softmax denom (per head pair, no DRAM bounce):
    sums rows -> sflat [1,1024] (ScalarE) -> SBUF scatter DMA [128,8] ->
    reciprocal_approx_fast (DVE) -> bf16 cast -> SBUF gather DMA [2,512] ->
    R = P2.T @ rec (PE broadcast) -> oT *= R (DVE, in place)
  yT = W_out.T @ oT + b  per 128-col chunk, each chunk DMAed out as done.

Scheduling: one software pipeline.  QKV e-tiles are loaded just-in-time
(q0/k0 of a sequence first, then v tiles, then the rest), the previous
sequence's output projection and the next sequence's QKV are interleaved
into the attention pairs so the TensorEngine never starves.  Input DMAs
are chunked (per e-tile / per dt) so the first matmul starts ~1.5us in.
"""

import os
import sys

import numpy as np

for _p in ("/opt/trn_rl_repo", "/root/.axon_site/_ro/trn_rl_repo"):
    if os.path.isdir(_p) and _p not in sys.path:
        sys.path.insert(0, _p)

import concourse.bass as bass  # noqa: E402
import concourse.mybir as mybir  # noqa: E402
from concourse import bacc  # noqa: E402
from concourse.tile import TileContext  # noqa: E402

F32 = mybir.dt.float32
BF16 = mybir.dt.bfloat16
F32R = mybir.dt.float32r

N_CORES = 8
SEQ_PER_CORE = 4  # (b*p)=32 sequences / 8 cores
N = 512  # tokens per sequence
D = 512  # model dim
HEADS = 8
DH = 64
SCALE = DH**-0.5
NT = N // 128  # 4 token tiles
DT = D // 128  # 4 dim tiles

EXP_F = mybir.ActivationFunctionType.Exp
COPY_F = mybir.ActivationFunctionType.Copy
IDENT_F = mybir.ActivationFunctionType.Identity
MULT = mybir.AluOpType.mult


def build_nc():
    """Build the per-core SPMD Bass program (same program on all 8 cores)."""
    nc = bacc.Bacc("TRN2", target_bir_lowering=False)

    xT = nc.declare_dram_parameter(
        "xT", [SEQ_PER_CORE, DT, 128, N], BF16, isOutput=False
    )
    # q e-tiles 0..3 then k e-tiles 0..3, each prepacked [128, DT, 128]
    wqk = nc.declare_dram_parameter("wqk", [8, 128, DT, 128], BF16, isOutput=False)
    wv = nc.declare_dram_parameter("wv", [128, DT, D], BF16, isOutput=False)
    wout = nc.declare_dram_parameter("wout", [128, DT, D], BF16, isOutput=False)
    bout = nc.declare_dram_parameter("bout", [D], F32, isOutput=False)
    p2d = nc.declare_dram_parameter("p2d", [1, 256], BF16, isOutput=False)
    out = nc.declare_dram_parameter(
        "out", [SEQ_PER_CORE, DT, 128, N], F32, isOutput=True
    )

    with TileContext(nc) as tc:
        with (
            tc.tile_pool(name="consts", bufs=1) as cpool,
            tc.tile_pool(name="xin", bufs=2) as xpool,
            tc.tile_pool(name="qk", bufs=2) as qkpool,
            tc.tile_pool(name="vaug", bufs=2) as vpool,
            tc.tile_pool(name="expt", bufs=3) as epool,
            tc.tile_pool(name="ot", bufs=2) as opool,
            tc.tile_pool(name="small", bufs=2) as spool,
            tc.tile_pool(name="yout", bufs=4) as ypool,
            tc.tile_pool(name="psq", bufs=2, space="PSUM") as psq,
            tc.tile_pool(name="psd", bufs=1, space="PSUM") as psd,
            tc.tile_pool(name="pso", bufs=1, space="PSUM") as pso,
        ):
            # ---- constants (DMA issue order matters: first-needed first;
            # all weight tensors host-prepacked so each DMA is contiguous) --
            wqk_sb = cpool.tile([128, 8, DT, 128], BF16, tag="wqk")

            seq_x = {}

            def x_alloc(s, interleave_et0=False):
                xts = [
                    xpool.tile([128, N], BF16, tag=f"x{dt}", name=f"x{s}_{dt}")
                    for dt in range(DT)
                ]
                for dt in range(DT):
                    # q0/k0 weight chunks interleaved per dt with x so the
                    # first matmul (q0, dt 0) starts after ~64KB has landed
                    if interleave_et0:
                        nc.sync.dma_start(wqk_sb[:, 0, dt], wqk[0, :, dt])
                    nc.sync.dma_start(xts[dt][:], xT[s, dt])
                seq_x[s] = xts

            x_alloc(0, interleave_et0=True)
            for dt in range(DT):
                nc.sync.dma_start(wqk_sb[:, 4, dt], wqk[4, :, dt])

            p2 = cpool.tile([1, 256], BF16, tag="p2")
            nc.sync.dma_start(p2[:], p2d[:])
            wv_sb = cpool.tile([128, DT, D], BF16, tag="wv")
            nc.sync.dma_start(wv_sb[:], wv[:])
            for et in (1, 5, 2, 6, 3, 7):
                nc.sync.dma_start(wqk_sb[:, et], wqk[et])
            wo_sb = cpool.tile([128, DT, D], BF16, tag="wo")
            nc.sync.dma_start(wo_sb[:], wout[:])
            b_sb = cpool.tile([128, DT], F32, tag="b")
            nc.sync.dma_start(b_sb[:], bout.rearrange("(t p) -> p t", p=128))

            seq_tiles = {}

            def qkv_alloc(s):
                q_sb = qkpool.tile([128, DT, N], BF16, tag="q", name=f"q{s}")
                k_sb = qkpool.tile([128, DT, N], BF16, tag="k", name=f"k{s}")
                vaug = vpool.tile(
                    [128, NT, HEADS, DH + 1], BF16, tag="v", name=f"v{s}"
                )
                nc.vector.memset(vaug[:, :, :, DH : DH + 1], 1.0)
                seq_tiles[s] = (q_sb, k_sb, vaug)

            def qkv_etile(s, kind, idx, pool_tag=None):
                """One QKV output tile: 4 accumulating matmuls + evacuation.
                kind 'q'/'k': e-tile idx (head pair idx); 'v': n-tile idx."""
                q_sb, k_sb, vaug = seq_tiles[s]
                xts = seq_x[s]
                if pool_tag is None:
                    ps = psq.tile(
                        [128, 512], F32, tag="ps", name=f"ps_{kind}{s}_{idx}"
                    )
                else:
                    ps = psd.tile(
                        [128, 512], F32, tag=pool_tag, name=f"ps_{kind}{s}_{idx}"
                    )
                if kind in ("q", "k"):
                    et = idx if kind == "q" else 4 + idx
                    for dt in range(DT):
                        nc.tensor.matmul(
                            ps[:],
                            lhsT=wqk_sb[:, et, dt, :],
                            rhs=xts[dt][:],
                            start=(dt == 0),
                            stop=(dt == DT - 1),
                        )
                    dest = q_sb if kind == "q" else k_sb
                    nc.vector.tensor_copy(dest[:, idx, :], ps[:])
                else:
                    nt = idx
                    for dt in range(DT):
                        nc.tensor.matmul(
                            ps[:],
                            lhsT=xts[dt][:, nt * 128 : (nt + 1) * 128],
                            rhs=wv_sb[:, dt, :],
                            start=(dt == 0),
                            stop=(dt == DT - 1),
                        )
                    nc.vector.tensor_copy(
                        vaug[:, nt, :, 0:DH],
                        ps.rearrange("p (h d) -> p h d", h=HEADS),
                    )

            rec2s = {}

            def norm_collect(s, t, oAB):
                """Sums rows -> 1/sums -> rec2 (bf16 [1,1024]), per pair.
                No DMA anywhere in the chain; the off-critical copies ride
                the idle Pool engine, except for the final pair whose chain
                is exposed at the kernel tail."""
                recf = spool.tile([1, 1024], F32, tag="recf", name=f"rf{s}_{t}")
                nc.scalar.activation(recf[0:1, :], oAB[64:65, :], COPY_F)
                nc.vector.reciprocal_approx_fast(recf[:], recf[:])
                rec2 = spool.tile([1, 1024], BF16, tag="rec2", name=f"r2_{s}_{t}")
                with nc.allow_low_precision(reason="softmax recip bf16"):
                    nc.vector.tensor_copy(rec2[:], recf[:])
                rec2s[(s, t)] = rec2

            def norm_pe(s, t):
                """R broadcast matmul + oT *= R for pair t of sequence s."""
                oT = seq_o[s]
                Rp = psq.tile([128, 512], F32, tag="ps", name=f"Rp{s}_{t}")
                rec2 = rec2s[(s, t)]
                nc.tensor.matmul(
                    Rp[:],
                    lhsT=p2[0:1, 0:128],
                    rhs=rec2[0:1, 0:512],
                    start=True,
                    stop=False,
                )
                nc.tensor.matmul(
                    Rp[:],
                    lhsT=p2[0:1, 128:256],
                    rhs=rec2[0:1, 512:1024],
                    start=False,
                    stop=True,
                )
                nc.vector.tensor_tensor(oT[:, t, :], oT[:, t, :], Rp[:], MULT)

            seq_o = {}

            def proj_dt(s, dt):
                """One 128-col chunk of the output projection + store."""
                oT = seq_o[s]
                ps = psq.tile([128, 512], F32, tag="ps", name=f"pj{s}_{dt}")
                for et in range(DT):
                    nc.tensor.matmul(
                        ps[:],
                        lhsT=wo_sb[:, et, dt * 128 : (dt + 1) * 128],
                        rhs=oT[:, et, :],
                        start=(et == 0),
                        stop=(et == DT - 1),
                    )
                yt = ypool.tile([128, 512], F32, tag="y", name=f"yt{s}_{dt}")
                nc.scalar.activation(yt[:], ps[:], IDENT_F, bias=b_sb[:, dt : dt + 1])
                nc.sync.dma_start(out[s, dt], yt[:])

            # ---- prologue: seq 0 q0/k0/v0/v1 -----------------------------
            qkv_alloc(0)
            qkv_etile(0, "q", 0, pool_tag="dA")
            qkv_etile(0, "k", 0, pool_tag="dB")
            qkv_etile(0, "v", 0)
            qkv_etile(0, "v", 1)

            for s in range(SEQ_PER_CORE):
                q_sb, k_sb, vaug = seq_tiles[s]
                oT = opool.tile([128, DT, N], BF16, tag="o", name=f"oT{s}")
                seq_o[s] = oT

                # filler units consumed inside the pair loop below.  Late
                # QKV e-tiles of sequence s ride in s's own stream (q2/k2
                # before pair 2, q3/k3 before pair 3); the early e-tiles of
                # s+1 (q0/k0 + all v) complete during s so s+1's pair 0 can
                # start immediately.
                fill = []

                def et_units(s_, pairs):
                    return [
                        (lambda k__=k, i__=i, s__=s_: qkv_etile(s__, k__, i__))
                        for (k, i) in pairs
                    ]

                if s == 0:
                    fill += et_units(0, [("v", 2), ("v", 3), ("q", 1), ("k", 1)])
                else:
                    fill += et_units(s, [("q", 2), ("k", 2)])
                    fill.append(lambda s_=s - 1: norm_pe(s_, 3))
                    fill += [
                        (lambda s_=s - 1, d_=d: proj_dt(s_, d_)) for d in range(DT)
                    ]
                if s == 0:
                    fill += et_units(0, [("q", 2), ("k", 2), ("q", 3), ("k", 3)])
                else:
                    fill += et_units(s, [("q", 3), ("k", 3)])
                if s + 1 < SEQ_PER_CORE:
                    qkv_alloc(s + 1)
                    fill += et_units(
                        s + 1,
                        [("q", 0), ("k", 0),
                         ("v", 0), ("v", 1), ("v", 2), ("v", 3),
                         ("q", 1), ("k", 1)],
                    )
                fi = 0

                def filler(k):
                    nonlocal fi
                    for _ in range(k):
                        if fi < len(fill):
                            fill[fi]()
                            fi += 1

                for t in range(4):  # head pair (2t, 2t+1)
                    if t == 1 and s + 1 < SEQ_PER_CORE:
                        x_alloc(s + 1)
                    expA = epool.tile([128, NT, N], BF16, tag="expA")
                    expB = epool.tile([128, NT, N], BF16, tag="expB")
                    oAB = pso.tile([128, 1024], F32, tag="oAB")

                    def dots(jh):
                        dA = psd.tile([128, 1024], F32, tag="dA")
                        dB = psd.tile([128, 1024], F32, tag="dB")
                        for jj in range(2):
                            jt = 2 * jh + jj
                            nc.tensor.matmul(
                                dA[:, jj * 512 : (jj + 1) * 512],
                                lhsT=k_sb[0:64, t, jt * 128 : (jt + 1) * 128],
                                rhs=q_sb[0:64, t, :],
                                start=True,
                                stop=True,
                            )
                            nc.tensor.matmul(
                                dB[:, jj * 512 : (jj + 1) * 512],
                                lhsT=k_sb[64:128, t, jt * 128 : (jt + 1) * 128],
                                rhs=q_sb[64:128, t, :],
                                start=True,
                                stop=True,
                            )
                        nc.scalar.activation(
                            expA[:, 2 * jh : 2 * jh + 2, :],
                            dA.rearrange("p (a n) -> p a n", a=2),
                            EXP_F,
                            scale=SCALE,
                        )
                        nc.scalar.activation(
                            expB[:, 2 * jh : 2 * jh + 2, :],
                            dB.rearrange("p (a n) -> p a n", a=2),
                            EXP_F,
                            scale=SCALE,
                        )

                    def attnv(jh):
                        for jj in range(2):
                            jt = 2 * jh + jj
                            nc.tensor.matmul(
                                oAB[0:65, 0:512],
                                lhsT=vaug[:, jt, 2 * t, :],
                                rhs=expA[:, jt, :],
                                start=(jt == 0),
                                stop=(jt == NT - 1),
                            )
                            nc.tensor.matmul(
                                oAB[0:65, 512:1024],
                                lhsT=vaug[:, jt, 2 * t + 1, :],
                                rhs=expB[:, jt, :],
                                start=(jt == 0),
                                stop=(jt == NT - 1),
                            )

                    dots(0)
                    filler(1)
                    dots(1)
                    filler(2)
                    attnv(0)
                    if t > 0:
                        norm_pe(s, t - 1)
                    filler(1)
                    attnv(1)

                    # evacuate unnormalized oT (f32 PSUM -> bf16 SBUF)
                    nc.vector.tensor_copy(oT[0:64, t, :], oAB[0:64, 0:512])
                    nc.vector.tensor_copy(oT[64:128, t, :], oAB[0:64, 512:1024])
                    norm_collect(s, t, oAB)

                    if s == SEQ_PER_CORE - 1 and t == 3:
                        # Final projection, split: e-tiles 0..2 accumulate
                        # into the PSUM banks the last exps just freed,
                        # running under pair 3's norm chain; only the et=3
                        # matmuls + bias + store remain after the last
                        # oT *= R.
                        pj01 = psd.tile([128, 1024], F32, tag="dA", name="pj01")
                        pj23 = psd.tile([128, 1024], F32, tag="dB", name="pj23")
                        for et in range(DT - 1):
                            for dt in range(DT):
                                pj = pj01 if dt < 2 else pj23
                                col = (dt % 2) * 512
                                nc.tensor.matmul(
                                    pj[:, col : col + 512],
                                    lhsT=wo_sb[:, et, dt * 128 : (dt + 1) * 128],
                                    rhs=oT[:, et, :],
                                    start=(et == 0),
                                    stop=False,
                                )

                # drain leftover fillers, then close out this sequence
                filler(len(fill))
                if s == SEQ_PER_CORE - 1:
                    norm_pe(s, 3)
                    for dt in range(DT):
                        pj = pj01 if dt < 2 else pj23
                        col = (dt % 2) * 512
                        nc.tensor.matmul(
                            pj[:, col : col + 512],
                            lhsT=wo_sb[:, DT - 1, dt * 128 : (dt + 1) * 128],
                            rhs=oT[:, DT - 1, :],
                            start=False,
                            stop=True,
                        )
                    for dt in range(DT):
                        pj = pj01 if dt < 2 else pj23
                        col = (dt % 2) * 512
                        yt = ypool.tile([128, 512], F32, tag="y", name=f"ytL{dt}")
                        eng = nc.vector if dt % 2 == 0 else nc.scalar
                        if eng is nc.vector:
                            eng.tensor_scalar_add(
                                yt[:], pj[:, col : col + 512], b_sb[:, dt : dt + 1]
                            )
                        else:
                            eng.activation(
                                yt[:],
                                pj[:, col : col + 512],
                                IDENT_F,
                                bias=b_sb[:, dt : dt + 1],
                            )
                        nc.sync.dma_start(out[s, dt], yt[:])

    nc.compile()
    return nc


def make_in_maps(x, W_qkv, W_out, b_out):
    """Shard + lay out full inputs into the 8 per-core input maps."""
    import ml_dtypes

    b, p, n, d = x.shape
    xs = np.ascontiguousarray(x, dtype=np.float32).reshape(b * p, n, d)
    Wb = (
        np.ascontiguousarray(W_qkv, dtype=np.float32)
        .reshape(DT, 128, 3, 4, 128)
        .astype(ml_dtypes.bfloat16)
    )
    # [8, 128, DT, 128] (partition-major): q e-tiles 0..3 then k e-tiles 0..3
    wqk = np.ascontiguousarray(
        np.concatenate(
            [Wb[:, :, 0].transpose(2, 1, 0, 3), Wb[:, :, 1].transpose(2, 1, 0, 3)]
        )
    )
    # [128, DT, D] (partition-major)
    wv = np.ascontiguousarray(Wb[:, :, 2].reshape(DT, 128, D).transpose(1, 0, 2))
    wout = np.ascontiguousarray(
        np.ascontiguousarray(W_out, dtype=np.float32)
        .reshape(DT, 128, D)
        .astype(ml_dtypes.bfloat16)
        .transpose(1, 0, 2)
    )
    bo = np.ascontiguousarray(b_out, dtype=np.float32)

    in_maps = []
    for c in range(N_CORES):
        seqs = xs[c * SEQ_PER_CORE : (c + 1) * SEQ_PER_CORE]  # (4, n, d)
        xT = (
            np.ascontiguousarray(seqs.transpose(0, 2, 1))
            .reshape(SEQ_PER_CORE, DT, 128, N)
            .astype(ml_dtypes.bfloat16)
        )
        p2 = np.zeros((1, 256), dtype=ml_dtypes.bfloat16)
        p2[0, 0:64] = 1.0
        p2[0, 128 + 64 : 256] = 1.0
        in_maps.append(
            {"xT": xT, "wqk": wqk, "wv": wv, "wout": wout, "bout": bo, "p2d": p2}
        )
    return in_maps


def assemble_output(results, b, p, n, d):
    """Gather per-core yT outputs back into the full (b,p,n,d) array."""
    y = np.empty((b * p, n, d), dtype=np.float32)
    for c in range(N_CORES):
        yT = np.asarray(results[c]["out"]).reshape(SEQ_PER_CORE, D, N)
        y[c * SEQ_PER_CORE : (c + 1) * SEQ_PER_CORE] = yT.transpose(0, 2, 1)
    return y.reshape(b, p, n, d)


_NC_CACHE = None


def _get_nc():
    global _NC_CACHE
    if _NC_CACHE is None:
        _NC_CACHE = build_nc()
    return _NC_CACHE


def run(inputs, trace=False, **spmd_kwargs):
    """Run on the 8 NeuronCores; returns (full_output, BassKernelResults)."""
    from concourse.bass_utils import run_bass_kernel_spmd

    x = np.asarray(inputs["x"])
    b, p, n, d = x.shape
    nc = _get_nc()
    in_maps = make_in_maps(x, inputs["W_qkv"], inputs["W_out"], inputs["b_out"])
    res = run_bass_kernel_spmd(
        nc, in_maps, core_ids=list(range(N_CORES)), trace=trace, **spmd_kwargs
    )
    return assemble_output(res.results, b, p, n, d), res


def kernel(x, W_qkv, W_out, b_out):
    out, _ = run({"x": x, "W_qkv": W_qkv, "W_out": W_out, "b_out": b_out})
    return out.astype(np.float32)


# revision 29
# speedup vs baseline: 1.3387x; 1.0134x over previous
"""Multi-head attention (b=2, p=16, n=512, d=512, h=8, dh=64) on 8 TRN2 cores.

Data-parallel over the 32 (b,p) sequences: 4 sequences per core, no
collectives.  Per-core dataflow (everything "T" = feature-on-partition):

  xT  (d,n)  --W_qkv stationary-->  qT,kT (e,n)   [e-tile = 2 heads]
  xT chunks stationary, W_v moving ->  v natural (n,e)  -> vaug (j,h,65)
  dotsT[j,i] = kT_h.T-slice @ qT_h   (K=64, heads A/B at rows 0:64/64:128)
  expT = exp(scale * dotsT)          (ScalarE, PSUM->SBUF, bf16 out)
  oT[dh,i] (+ sums row 64) = vaug_h.T @ expT_h   (M=65, ones column -> sums)
  softmax denom (per head pair, no DRAM bounce):
    sums rows -> sflat [1,1024] (ScalarE) -> SBUF scatter DMA [128,8] ->
    reciprocal_approx_fast (DVE) -> bf16 cast -> SBUF gather DMA [2,512] ->
    R = P2.T @ rec (PE broadcast) -> oT *= R (DVE, in place)
  yT = W_out.T @ oT + b  per 128-col chunk, each chunk DMAed out as done.

Scheduling: one software pipeline.  QKV e-tiles are loaded just-in-time
(q0/k0 of a sequence first, then v tiles, then the rest), the previous
sequence's output projection and the next sequence's QKV are interleaved
into the attention pairs so the TensorEngine never starves.  Input DMAs
are chunked (per e-tile / per dt) so the first matmul starts ~1.5us in.
"""

import os
import sys

import numpy as np

for _p in ("/opt/trn_rl_repo", "/root/.axon_site/_ro/trn_rl_repo"):
    if os.path.isdir(_p) and _p not in sys.path:
        sys.path.insert(0, _p)

import concourse.bass as bass  # noqa: E402
import concourse.mybir as mybir  # noqa: E402
from concourse import bacc  # noqa: E402
from concourse.tile import TileContext  # noqa: E402

F32 = mybir.dt.float32
BF16 = mybir.dt.bfloat16
F32R = mybir.dt.float32r

N_CORES = 8
SEQ_PER_CORE = 4  # (b*p)=32 sequences / 8 cores
N = 512  # tokens per sequence
D = 512  # model dim
HEADS = 8
DH = 64
SCALE = DH**-0.5
NT = N // 128  # 4 token tiles
DT = D // 128  # 4 dim tiles

EXP_F = mybir.ActivationFunctionType.Exp
COPY_F = mybir.ActivationFunctionType.Copy
IDENT_F = mybir.ActivationFunctionType.Identity
MULT = mybir.AluOpType.mult


def build_nc():
    """Build the per-core SPMD Bass program (same program on all 8 cores)."""
    nc = bacc.Bacc("TRN2", target_bir_lowering=False)

    xT = nc.declare_dram_parameter(
        "xT", [SEQ_PER_CORE, DT, 128, N], BF16, isOutput=False
    )
    # q e-tiles 0..3 then k e-tiles 0..3, each prepacked [128, DT, 128]
    wqk = nc.declare_dram_parameter("wqk", [8, 128, DT, 128], BF16, isOutput=False)
    wv = nc.declare_dram_parameter("wv", [128, DT, D], BF16, isOutput=False)
    wout = nc.declare_dram_parameter("wout", [128, DT, D], BF16, isOutput=False)
    bout = nc.declare_dram_parameter("bout", [D], F32, isOutput=False)
    p2d = nc.declare_dram_parameter("p2d", [1, 256], BF16, isOutput=False)
    out = nc.declare_dram_parameter(
        "out", [SEQ_PER_CORE, DT, 128, N], F32, isOutput=True
    )

    with TileContext(nc) as tc:
        with (
            tc.tile_pool(name="consts", bufs=1) as cpool,
            tc.tile_pool(name="xin", bufs=2) as xpool,
            tc.tile_pool(name="qk", bufs=2) as qkpool,
            tc.tile_pool(name="vaug", bufs=2) as vpool,
            tc.tile_pool(name="expt", bufs=3) as epool,
            tc.tile_pool(name="ot", bufs=2) as opool,
            tc.tile_pool(name="small", bufs=2) as spool,
            tc.tile_pool(name="yout", bufs=4) as ypool,
            tc.tile_pool(name="psq", bufs=2, space="PSUM") as psq,
            tc.tile_pool(name="psd", bufs=1, space="PSUM") as psd,
            tc.tile_pool(name="pso", bufs=1, space="PSUM") as pso,
        ):
            # ---- constants (DMA issue order matters: first-needed first;
            # all weight tensors host-prepacked so each DMA is contiguous) --
            wqk_sb = cpool.tile([128, 8, DT, 128], BF16, tag="wqk")

            seq_x = {}

            def x_alloc(s, interleave_et0=False):
                x2 = [
                    xpool.tile([128, 2, N], BF16, tag=f"x{h}", name=f"x{s}_{h}")
                    for h in range(2)
                ]
                for h in range(2):
                    # q0/k0 weight chunks interleaved with x so the first
                    # matmuls start as soon as their operands land
                    if interleave_et0:
                        nc.sync.dma_start(
                            wqk_sb[:, 0, 2 * h : 2 * h + 2],
                            wqk[0, :, 2 * h : 2 * h + 2],
                        )
                    nc.sync.dma_start(
                        x2[h][:], xT[s, 2 * h : 2 * h + 2].rearrange("t p n -> p t n")
                    )
                seq_x[s] = [x2[0][:, 0], x2[0][:, 1], x2[1][:, 0], x2[1][:, 1]]

            x_alloc(0, interleave_et0=True)
            nc.sync.dma_start(wqk_sb[:, 4], wqk[4])

            p2 = cpool.tile([1, 256], BF16, tag="p2")
            nc.sync.dma_start(p2[:], p2d[:])
            wv_sb = cpool.tile([128, DT, D], BF16, tag="wv")
            nc.sync.dma_start(wv_sb[:], wv[:])
            for et in (1, 5, 2, 6, 3, 7):
                nc.sync.dma_start(wqk_sb[:, et], wqk[et])
            wo_sb = cpool.tile([128, DT, D], BF16, tag="wo")
            nc.sync.dma_start(wo_sb[:], wout[:])
            b_sb = cpool.tile([128, DT], F32, tag="b")
            nc.sync.dma_start(b_sb[:], bout.rearrange("(t p) -> p t", p=128))

            seq_tiles = {}

            def qkv_alloc(s):
                q_sb = qkpool.tile([128, DT, N], BF16, tag="q", name=f"q{s}")
                k_sb = qkpool.tile([128, DT, N], BF16, tag="k", name=f"k{s}")
                vaug = vpool.tile(
                    [128, NT, HEADS, DH + 1], BF16, tag="v", name=f"v{s}"
                )
                nc.vector.memset(vaug[:, :, :, DH : DH + 1], 1.0)
                seq_tiles[s] = (q_sb, k_sb, vaug)

            def qkv_etile(s, kind, idx, pool_tag=None):
                """One QKV output tile: 4 accumulating matmuls + evacuation.
                kind 'q'/'k': e-tile idx (head pair idx); 'v': n-tile idx."""
                q_sb, k_sb, vaug = seq_tiles[s]
                xts = seq_x[s]
                if pool_tag is None:
                    ps = psq.tile(
                        [128, 512], F32, tag="ps", name=f"ps_{kind}{s}_{idx}"
                    )
                else:
                    ps = psd.tile(
                        [128, 512], F32, tag=pool_tag, name=f"ps_{kind}{s}_{idx}"
                    )
                if kind in ("q", "k"):
                    et = idx if kind == "q" else 4 + idx
                    for dt in range(DT):
                        nc.tensor.matmul(
                            ps[:],
                            lhsT=wqk_sb[:, et, dt, :],
                            rhs=xts[dt][:],
                            start=(dt == 0),
                            stop=(dt == DT - 1),
                        )
                    dest = q_sb if kind == "q" else k_sb
                    nc.vector.tensor_copy(dest[:, idx, :], ps[:])
                else:
                    nt = idx
                    for dt in range(DT):
                        nc.tensor.matmul(
                            ps[:],
                            lhsT=xts[dt][:, nt * 128 : (nt + 1) * 128],
                            rhs=wv_sb[:, dt, :],
                            start=(dt == 0),
                            stop=(dt == DT - 1),
                        )
                    nc.vector.tensor_copy(
                        vaug[:, nt, :, 0:DH],
                        ps.rearrange("p (h d) -> p h d", h=HEADS),
                    )

            rec2s = {}

            def norm_collect(s, t, oAB):
                """Sums rows -> 1/sums -> rec2 (bf16 [1,1024]), per pair.
                No DMA anywhere in the chain; the off-critical copies ride
                the idle Pool engine, except for the final pair whose chain
                is exposed at the kernel tail."""
                recf = spool.tile([1, 1024], F32, tag="recf", name=f"rf{s}_{t}")
                nc.scalar.activation(recf[0:1, :], oAB[64:65, :], COPY_F)
                nc.vector.reciprocal_approx_fast(recf[:], recf[:])
                rec2 = spool.tile([1, 1024], BF16, tag="rec2", name=f"r2_{s}_{t}")
                with nc.allow_low_precision(reason="softmax recip bf16"):
                    nc.vector.tensor_copy(rec2[:], recf[:])
                rec2s[(s, t)] = rec2

            def norm_pe(s, t):
                """R broadcast matmul + oT *= R for pair t of sequence s."""
                oT = seq_o[s]
                Rp = psq.tile([128, 512], F32, tag="ps", name=f"Rp{s}_{t}")
                rec2 = rec2s[(s, t)]
                nc.tensor.matmul(
                    Rp[:],
                    lhsT=p2[0:1, 0:128],
                    rhs=rec2[0:1, 0:512],
                    start=True,
                    stop=False,
                )
                nc.tensor.matmul(
                    Rp[:],
                    lhsT=p2[0:1, 128:256],
                    rhs=rec2[0:1, 512:1024],
                    start=False,
                    stop=True,
                )
                nc.vector.tensor_tensor(oT[:, t, :], oT[:, t, :], Rp[:], MULT)

            seq_o = {}

            def proj_dt(s, dt):
                """One 128-col chunk of the output projection + store."""
                oT = seq_o[s]
                ps = psq.tile([128, 512], F32, tag="ps", name=f"pj{s}_{dt}")
                for et in range(DT):
                    nc.tensor.matmul(
                        ps[:],
                        lhsT=wo_sb[:, et, dt * 128 : (dt + 1) * 128],
                        rhs=oT[:, et, :],
                        start=(et == 0),
                        stop=(et == DT - 1),
                    )
                yt = ypool.tile([128, 512], F32, tag="y", name=f"yt{s}_{dt}")
                nc.scalar.activation(yt[:], ps[:], IDENT_F, bias=b_sb[:, dt : dt + 1])
                nc.sync.dma_start(out[s, dt], yt[:])

            # ---- prologue: seq 0 q0/k0/v0/v1 -----------------------------
            qkv_alloc(0)
            qkv_etile(0, "q", 0, pool_tag="dA")
            qkv_etile(0, "k", 0, pool_tag="dB")
            qkv_etile(0, "v", 0)
            qkv_etile(0, "v", 1)

            for s in range(SEQ_PER_CORE):
                q_sb, k_sb, vaug = seq_tiles[s]
                oT = opool.tile([128, DT, N], BF16, tag="o", name=f"oT{s}")
                seq_o[s] = oT

                # filler units consumed inside the pair loop below.  Late
                # QKV e-tiles of sequence s ride in s's own stream (q2/k2
                # before pair 2, q3/k3 before pair 3); the early e-tiles of
                # s+1 (q0/k0 + all v) complete during s so s+1's pair 0 can
                # start immediately.
                fill = []

                def et_units(s_, pairs):
                    return [
                        (lambda k__=k, i__=i, s__=s_: qkv_etile(s__, k__, i__))
                        for (k, i) in pairs
                    ]

                if s == 0:
                    fill += et_units(0, [("v", 2), ("v", 3), ("q", 1), ("k", 1)])
                else:
                    fill += et_units(s, [("q", 2), ("k", 2)])
                    fill.append(lambda s_=s - 1: norm_pe(s_, 3))
                    fill += [
                        (lambda s_=s - 1, d_=d: proj_dt(s_, d_)) for d in range(DT)
                    ]
                if s == 0:
                    fill += et_units(0, [("q", 2), ("k", 2), ("q", 3), ("k", 3)])
                else:
                    fill += et_units(s, [("q", 3), ("k", 3)])
                if s + 1 < SEQ_PER_CORE:
                    qkv_alloc(s + 1)
                    fill += et_units(
                        s + 1,
                        [("q", 0), ("k", 0),
                         ("v", 0), ("v", 1), ("v", 2), ("v", 3),
                         ("q", 1), ("k", 1)],
                    )
                fi = 0

                def filler(k):
                    nonlocal fi
                    for _ in range(k):
                        if fi < len(fill):
                            fill[fi]()
                            fi += 1

                for t in range(4):  # head pair (2t, 2t+1)
                    if t == 1 and s + 1 < SEQ_PER_CORE:
                        x_alloc(s + 1)
                    expA = epool.tile([128, NT, N], BF16, tag="expA")
                    expB = epool.tile([128, NT, N], BF16, tag="expB")
                    oAB = pso.tile([128, 1024], F32, tag="oAB")

                    def dots(jh):
                        dA = psd.tile([128, 1024], F32, tag="dA")
                        dB = psd.tile([128, 1024], F32, tag="dB")
                        for jj in range(2):
                            jt = 2 * jh + jj
                            nc.tensor.matmul(
                                dA[:, jj * 512 : (jj + 1) * 512],
                                lhsT=k_sb[0:64, t, jt * 128 : (jt + 1) * 128],
                                rhs=q_sb[0:64, t, :],
                                start=True,
                                stop=True,
                            )
                            nc.tensor.matmul(
                                dB[:, jj * 512 : (jj + 1) * 512],
                                lhsT=k_sb[64:128, t, jt * 128 : (jt + 1) * 128],
                                rhs=q_sb[64:128, t, :],
                                start=True,
                                stop=True,
                            )
                        nc.scalar.activation(
                            expA[:, 2 * jh : 2 * jh + 2, :],
                            dA.rearrange("p (a n) -> p a n", a=2),
                            EXP_F,
                            scale=SCALE,
                        )
                        nc.scalar.activation(
                            expB[:, 2 * jh : 2 * jh + 2, :],
                            dB.rearrange("p (a n) -> p a n", a=2),
                            EXP_F,
                            scale=SCALE,
                        )

                    def attnv(jh):
                        for jj in range(2):
                            jt = 2 * jh + jj
                            nc.tensor.matmul(
                                oAB[0:65, 0:512],
                                lhsT=vaug[:, jt, 2 * t, :],
                                rhs=expA[:, jt, :],
                                start=(jt == 0),
                                stop=(jt == NT - 1),
                            )
                            nc.tensor.matmul(
                                oAB[0:65, 512:1024],
                                lhsT=vaug[:, jt, 2 * t + 1, :],
                                rhs=expB[:, jt, :],
                                start=(jt == 0),
                                stop=(jt == NT - 1),
                            )

                    dots(0)
                    filler(1)
                    dots(1)
                    filler(2)
                    attnv(0)
                    if t > 0:
                        norm_pe(s, t - 1)
                    filler(1)
                    attnv(1)

                    # evacuate unnormalized oT (f32 PSUM -> bf16 SBUF)
                    nc.vector.tensor_copy(oT[0:64, t, :], oAB[0:64, 0:512])
                    nc.vector.tensor_copy(oT[64:128, t, :], oAB[0:64, 512:1024])
                    norm_collect(s, t, oAB)

                    if s == SEQ_PER_CORE - 1 and t == 3:
                        # Final projection, split: e-tiles 0..2 accumulate
                        # into the PSUM banks the last exps just freed,
                        # running under pair 3's norm chain; only the et=3
                        # matmuls + bias + store remain after the last
                        # oT *= R.
                        pj01 = psd.tile([128, 1024], F32, tag="dA", name="pj01")
                        pj23 = psd.tile([128, 1024], F32, tag="dB", name="pj23")
                        for et in range(DT - 1):
                            for dt in range(DT):
                                pj = pj01 if dt < 2 else pj23
                                col = (dt % 2) * 512
                                nc.tensor.matmul(
                                    pj[:, col : col + 512],
                                    lhsT=wo_sb[:, et, dt * 128 : (dt + 1) * 128],
                                    rhs=oT[:, et, :],
                                    start=(et == 0),
                                    stop=False,
                                )

                # drain leftover fillers, then close out this sequence
                filler(len(fill))
                if s == SEQ_PER_CORE - 1:
                    norm_pe(s, 3)
                    for dt in range(DT):
                        pj = pj01 if dt < 2 else pj23
                        col = (dt % 2) * 512
                        nc.tensor.matmul(
                            pj[:, col : col + 512],
                            lhsT=wo_sb[:, DT - 1, dt * 128 : (dt + 1) * 128],
                            rhs=oT[:, DT - 1, :],
                            start=False,
                            stop=True,
                        )
                    for dt in range(DT):
                        pj = pj01 if dt < 2 else pj23
                        col = (dt % 2) * 512
                        yt = ypool.tile([128, 512], F32, tag="y", name=f"ytL{dt}")
                        eng = nc.vector if dt % 2 == 0 else nc.scalar
                        if eng is nc.vector:
                            eng.tensor_scalar_add(
                                yt[:], pj[:, col : col + 512], b_sb[:, dt : dt + 1]
                            )
                        else:
                            eng.activation(
                                yt[:],
                                pj[:, col : col + 512],
                                IDENT_F,
                                bias=b_sb[:, dt : dt + 1],
                            )
                        nc.sync.dma_start(out[s, dt], yt[:])

    nc.compile()
    return nc


def make_in_maps(x, W_qkv, W_out, b_out):
    """Shard + lay out full inputs into the 8 per-core input maps."""
    import ml_dtypes

    b, p, n, d = x.shape
    xs = np.ascontiguousarray(x, dtype=np.float32).reshape(b * p, n, d)
    Wb = (
        np.ascontiguousarray(W_qkv, dtype=np.float32)
        .reshape(DT, 128, 3, 4, 128)
        .astype(ml_dtypes.bfloat16)
    )
    # [8, 128, DT, 128] (partition-major): q e-tiles 0..3 then k e-tiles 0..3
    wqk = np.ascontiguousarray(
        np.concatenate(
            [Wb[:, :, 0].transpose(2, 1, 0, 3), Wb[:, :, 1].transpose(2, 1, 0, 3)]
        )
    )
    # [128, DT, D] (partition-major)
    wv = np.ascontiguousarray(Wb[:, :, 2].reshape(DT, 128, D).transpose(1, 0, 2))
    wout = np.ascontiguousarray(
        np.ascontiguousarray(W_out, dtype=np.float32)
        .reshape(DT, 128, D)
        .astype(ml_dtypes.bfloat16)
        .transpose(1, 0, 2)
    )
    bo = np.ascontiguousarray(b_out, dtype=np.float32)

    in_maps = []
    for c in range(N_CORES):
        seqs = xs[c * SEQ_PER_CORE : (c + 1) * SEQ_PER_CORE]  # (4, n, d)
        xT = (
            np.ascontiguousarray(seqs.transpose(0, 2, 1))
            .reshape(SEQ_PER_CORE, DT, 128, N)
            .astype(ml_dtypes.bfloat16)
        )
        p2 = np.zeros((1, 256), dtype=ml_dtypes.bfloat16)
        p2[0, 0:64] = 1.0
        p2[0, 128 + 64 : 256] = 1.0
        in_maps.append(
            {"xT": xT, "wqk": wqk, "wv": wv, "wout": wout, "bout": bo, "p2d": p2}
        )
    return in_maps


def assemble_output(results, b, p, n, d):
    """Gather per-core yT outputs back into the full (b,p,n,d) array."""
    y = np.empty((b * p, n, d), dtype=np.float32)
    for c in range(N_CORES):
        yT = np.asarray(results[c]["out"]).reshape(SEQ_PER_CORE, D, N)
        y[c * SEQ_PER_CORE : (c + 1) * SEQ_PER_CORE] = yT.transpose(0, 2, 1)
    return y.reshape(b, p, n, d)


_NC_CACHE = None


def _get_nc():
    global _NC_CACHE
    if _NC_CACHE is None:
        _NC_CACHE = build_nc()
    return _NC_CACHE


def run(inputs, trace=False, **spmd_kwargs):
    """Run on the 8 NeuronCores; returns (full_output, BassKernelResults)."""
    from concourse.bass_utils import run_bass_kernel_spmd

    x = np.asarray(inputs["x"])
    b, p, n, d = x.shape
    nc = _get_nc()
    in_maps = make_in_maps(x, inputs["W_qkv"], inputs["W_out"], inputs["b_out"])
    res = run_bass_kernel_spmd(
        nc, in_maps, core_ids=list(range(N_CORES)), trace=trace, **spmd_kwargs
    )
    return assemble_output(res.results, b, p, n, d), res


def kernel(x, W_qkv, W_out, b_out):
    out, _ = run({"x": x, "W_qkv": W_qkv, "W_out": W_out, "b_out": b_out})
    return out.astype(np.float32)


# revision 30
# speedup vs baseline: 1.3405x; 1.0014x over previous
"""Multi-head attention (b=2, p=16, n=512, d=512, h=8, dh=64) on 8 TRN2 cores.

Data-parallel over the 32 (b,p) sequences: 4 sequences per core, no
collectives.  Per-core dataflow (everything "T" = feature-on-partition):

  xT  (d,n)  --W_qkv stationary-->  qT,kT (e,n)   [e-tile = 2 heads]
  xT chunks stationary, W_v moving ->  v natural (n,e)  -> vaug (j,h,65)
  dotsT[j,i] = kT_h.T-slice @ qT_h   (K=64, heads A/B at rows 0:64/64:128)
  expT = exp(scale * dotsT)          (ScalarE, PSUM->SBUF, bf16 out)
  oT[dh,i] (+ sums row 64) = vaug_h.T @ expT_h   (M=65, ones column -> sums)
  softmax denom (per head pair, zero DMA):
    sums row [1,1024] -> SBUF (one ScalarE copy) -> reciprocal_approx_fast
    in place (DVE) -> bf16 cast (DVE) -> two accumulating K=1 mask matmuls
    broadcast A/B recips across partitions -> oT *= R (DVE, in place)
  yT = W_out.T @ oT, bias added via ScalarE Identity-activation, each
  128-col chunk DMAed out as soon as it is done.

Scheduling: one software pipeline.  QKV e-tiles are computed just-in-time
(a sequence's q0/k0/v tiles complete during the previous sequence, its
late q/k e-tiles ride its own early pairs), the previous sequence's
output projection and the next sequence's QKV fill the attention pairs so
the TensorEngine never starves.  Input DMAs are chunked so the first
matmul starts right after the framework preamble.  The final projection
accumulates e-tiles 0..2 into PSUM banks freed by the last exps while the
last softmax-denominator chain runs, leaving only 4 matmuls + bias +
store after the final normalization.
"""

import os
import sys

import numpy as np

for _p in ("/opt/trn_rl_repo", "/root/.axon_site/_ro/trn_rl_repo"):
    if os.path.isdir(_p) and _p not in sys.path:
        sys.path.insert(0, _p)

import concourse.bass as bass  # noqa: E402
import concourse.mybir as mybir  # noqa: E402
from concourse import bacc  # noqa: E402
from concourse.tile import TileContext  # noqa: E402

F32 = mybir.dt.float32
BF16 = mybir.dt.bfloat16
F32R = mybir.dt.float32r

N_CORES = 8
SEQ_PER_CORE = 4  # (b*p)=32 sequences / 8 cores
N = 512  # tokens per sequence
D = 512  # model dim
HEADS = 8
DH = 64
SCALE = DH**-0.5
NT = N // 128  # 4 token tiles
DT = D // 128  # 4 dim tiles

EXP_F = mybir.ActivationFunctionType.Exp
COPY_F = mybir.ActivationFunctionType.Copy
IDENT_F = mybir.ActivationFunctionType.Identity
MULT = mybir.AluOpType.mult


def build_nc():
    """Build the per-core SPMD Bass program (same program on all 8 cores)."""
    nc = bacc.Bacc("TRN2", target_bir_lowering=False)

    xT = nc.declare_dram_parameter(
        "xT", [SEQ_PER_CORE, DT, 128, N], BF16, isOutput=False
    )
    # q e-tiles 0..3 then k e-tiles 0..3, each prepacked [128, DT, 128]
    wqk = nc.declare_dram_parameter("wqk", [8, 128, DT, 128], BF16, isOutput=False)
    wv = nc.declare_dram_parameter("wv", [128, DT, D], BF16, isOutput=False)
    wout = nc.declare_dram_parameter("wout", [128, DT, D], BF16, isOutput=False)
    bout = nc.declare_dram_parameter("bout", [D], F32, isOutput=False)
    p2d = nc.declare_dram_parameter("p2d", [1, 256], BF16, isOutput=False)
    out = nc.declare_dram_parameter(
        "out", [SEQ_PER_CORE, DT, 128, N], F32, isOutput=True
    )

    with TileContext(nc) as tc:
        with (
            tc.tile_pool(name="consts", bufs=1) as cpool,
            tc.tile_pool(name="xin", bufs=2) as xpool,
            tc.tile_pool(name="qk", bufs=2) as qkpool,
            tc.tile_pool(name="vaug", bufs=2) as vpool,
            tc.tile_pool(name="expt", bufs=3) as epool,
            tc.tile_pool(name="ot", bufs=2) as opool,
            tc.tile_pool(name="small", bufs=2) as spool,
            tc.tile_pool(name="yout", bufs=4) as ypool,
            tc.tile_pool(name="psq", bufs=2, space="PSUM") as psq,
            tc.tile_pool(name="psd", bufs=1, space="PSUM") as psd,
            tc.tile_pool(name="pso", bufs=1, space="PSUM") as pso,
        ):
            # ---- constants (DMA issue order matters: first-needed first;
            # all weight tensors host-prepacked so each DMA is contiguous) --
            wqk_sb = cpool.tile([128, 8, DT, 128], BF16, tag="wqk")

            seq_x = {}

            def x_alloc(s, interleave_et0=False):
                x2 = [
                    xpool.tile([128, 2, N], BF16, tag=f"x{h}", name=f"x{s}_{h}")
                    for h in range(2)
                ]
                for h in range(2):
                    # q0/k0 weight chunks interleaved with x so the first
                    # matmuls start as soon as their operands land
                    if interleave_et0:
                        nc.sync.dma_start(
                            wqk_sb[:, 0, 2 * h : 2 * h + 2],
                            wqk[0, :, 2 * h : 2 * h + 2],
                        )
                    nc.sync.dma_start(
                        x2[h][:], xT[s, 2 * h : 2 * h + 2].rearrange("t p n -> p t n")
                    )
                seq_x[s] = [x2[0][:, 0], x2[0][:, 1], x2[1][:, 0], x2[1][:, 1]]

            x_alloc(0, interleave_et0=True)
            nc.sync.dma_start(wqk_sb[:, 4], wqk[4])

            p2 = cpool.tile([1, 256], BF16, tag="p2")
            nc.sync.dma_start(p2[:], p2d[:])
            wv_sb = cpool.tile([128, DT, D], BF16, tag="wv")
            nc.sync.dma_start(wv_sb[:], wv[:])
            for et in (1, 5, 2, 6, 3, 7):
                nc.sync.dma_start(wqk_sb[:, et], wqk[et])
            wo_sb = cpool.tile([128, DT, D], BF16, tag="wo")
            nc.sync.dma_start(wo_sb[:], wout[:])
            b_sb = cpool.tile([128, DT], F32, tag="b")
            nc.sync.dma_start(b_sb[:], bout.rearrange("(t p) -> p t", p=128))

            seq_tiles = {}

            def qkv_alloc(s):
                q_sb = qkpool.tile([128, DT, N], BF16, tag="q", name=f"q{s}")
                k_sb = qkpool.tile([128, DT, N], BF16, tag="k", name=f"k{s}")
                vaug = vpool.tile(
                    [128, NT, HEADS, DH + 1], BF16, tag="v", name=f"v{s}"
                )
                nc.vector.memset(vaug[:, :, :, DH : DH + 1], 1.0)
                seq_tiles[s] = (q_sb, k_sb, vaug)

            def qkv_etile(s, kind, idx, pool_tag=None):
                """One QKV output tile: 4 accumulating matmuls + evacuation.
                kind 'q'/'k': e-tile idx (head pair idx); 'v': n-tile idx."""
                q_sb, k_sb, vaug = seq_tiles[s]
                xts = seq_x[s]
                if pool_tag is None:
                    ps = psq.tile(
                        [128, 512], F32, tag="ps", name=f"ps_{kind}{s}_{idx}"
                    )
                else:
                    ps = psd.tile(
                        [128, 512], F32, tag=pool_tag, name=f"ps_{kind}{s}_{idx}"
                    )
                if kind in ("q", "k"):
                    et = idx if kind == "q" else 4 + idx
                    for dt in range(DT):
                        nc.tensor.matmul(
                            ps[:],
                            lhsT=wqk_sb[:, et, dt, :],
                            rhs=xts[dt][:],
                            start=(dt == 0),
                            stop=(dt == DT - 1),
                        )
                    dest = q_sb if kind == "q" else k_sb
                    nc.vector.tensor_copy(dest[:, idx, :], ps[:])
                else:
                    nt = idx
                    for dt in range(DT):
                        nc.tensor.matmul(
                            ps[:],
                            lhsT=xts[dt][:, nt * 128 : (nt + 1) * 128],
                            rhs=wv_sb[:, dt, :],
                            start=(dt == 0),
                            stop=(dt == DT - 1),
                        )
                    nc.vector.tensor_copy(
                        vaug[:, nt, :, 0:DH],
                        ps.rearrange("p (h d) -> p h d", h=HEADS),
                    )

            rec2s = {}

            def norm_collect(s, t, oAB):
                """Sums rows -> 1/sums -> rec2 (bf16 [1,1024]), per pair.
                No DMA anywhere in the chain; the off-critical copies ride
                the idle Pool engine, except for the final pair whose chain
                is exposed at the kernel tail."""
                recf = spool.tile([1, 1024], F32, tag="recf", name=f"rf{s}_{t}")
                nc.scalar.activation(recf[0:1, :], oAB[64:65, :], COPY_F)
                nc.vector.reciprocal_approx_fast(recf[:], recf[:])
                rec2 = spool.tile([1, 1024], BF16, tag="rec2", name=f"r2_{s}_{t}")
                with nc.allow_low_precision(reason="softmax recip bf16"):
                    nc.vector.tensor_copy(rec2[:], recf[:])
                rec2s[(s, t)] = rec2

            def norm_pe(s, t):
                """R broadcast matmul + oT *= R for pair t of sequence s."""
                oT = seq_o[s]
                Rp = psq.tile([128, 512], F32, tag="ps", name=f"Rp{s}_{t}")
                rec2 = rec2s[(s, t)]
                nc.tensor.matmul(
                    Rp[:],
                    lhsT=p2[0:1, 0:128],
                    rhs=rec2[0:1, 0:512],
                    start=True,
                    stop=False,
                )
                nc.tensor.matmul(
                    Rp[:],
                    lhsT=p2[0:1, 128:256],
                    rhs=rec2[0:1, 512:1024],
                    start=False,
                    stop=True,
                )
                nc.vector.tensor_tensor(oT[:, t, :], oT[:, t, :], Rp[:], MULT)

            seq_o = {}

            def proj_dt(s, dt):
                """One 128-col chunk of the output projection + store."""
                oT = seq_o[s]
                ps = psq.tile([128, 512], F32, tag="ps", name=f"pj{s}_{dt}")
                for et in range(DT):
                    nc.tensor.matmul(
                        ps[:],
                        lhsT=wo_sb[:, et, dt * 128 : (dt + 1) * 128],
                        rhs=oT[:, et, :],
                        start=(et == 0),
                        stop=(et == DT - 1),
                    )
                yt = ypool.tile([128, 512], F32, tag="y", name=f"yt{s}_{dt}")
                nc.scalar.activation(yt[:], ps[:], IDENT_F, bias=b_sb[:, dt : dt + 1])
                nc.sync.dma_start(out[s, dt], yt[:])

            # ---- prologue: seq 0 q0/k0/v0/v1 -----------------------------
            qkv_alloc(0)
            qkv_etile(0, "q", 0, pool_tag="dA")
            qkv_etile(0, "k", 0, pool_tag="dB")
            qkv_etile(0, "v", 0)
            qkv_etile(0, "v", 1)

            for s in range(SEQ_PER_CORE):
                q_sb, k_sb, vaug = seq_tiles[s]
                oT = opool.tile([128, DT, N], BF16, tag="o", name=f"oT{s}")
                seq_o[s] = oT

                # filler units consumed inside the pair loop below.  Late
                # QKV e-tiles of sequence s ride in s's own stream (q2/k2
                # before pair 2, q3/k3 before pair 3); the early e-tiles of
                # s+1 (q0/k0 + all v) complete during s so s+1's pair 0 can
                # start immediately.
                fill = []

                def et_units(s_, pairs):
                    return [
                        (lambda k__=k, i__=i, s__=s_: qkv_etile(s__, k__, i__))
                        for (k, i) in pairs
                    ]

                if s == 0:
                    fill += et_units(0, [("v", 2), ("v", 3), ("q", 1), ("k", 1)])
                else:
                    fill += et_units(s, [("q", 2), ("k", 2)])
                    fill.append(lambda s_=s - 1: norm_pe(s_, 3))
                    fill += [
                        (lambda s_=s - 1, d_=d: proj_dt(s_, d_)) for d in range(DT)
                    ]
                if s == 0:
                    fill += et_units(0, [("q", 2), ("k", 2), ("q", 3), ("k", 3)])
                else:
                    fill += et_units(s, [("q", 3), ("k", 3)])
                if s + 1 < SEQ_PER_CORE:
                    qkv_alloc(s + 1)
                    fill += et_units(
                        s + 1,
                        [("q", 0), ("k", 0),
                         ("v", 0), ("v", 1), ("v", 2), ("v", 3),
                         ("q", 1), ("k", 1)],
                    )
                fi = 0

                def filler(k):
                    nonlocal fi
                    for _ in range(k):
                        if fi < len(fill):
                            fill[fi]()
                            fi += 1

                for t in range(4):  # head pair (2t, 2t+1)
                    if t == 1 and s + 1 < SEQ_PER_CORE:
                        x_alloc(s + 1)
                    expA = epool.tile([128, NT, N], BF16, tag="expA")
                    expB = epool.tile([128, NT, N], BF16, tag="expB")
                    oAB = pso.tile([128, 1024], F32, tag="oAB")

                    def dots(jh):
                        dA = psd.tile([128, 1024], F32, tag="dA")
                        dB = psd.tile([128, 1024], F32, tag="dB")
                        for jj in range(2):
                            jt = 2 * jh + jj
                            nc.tensor.matmul(
                                dA[:, jj * 512 : (jj + 1) * 512],
                                lhsT=k_sb[0:64, t, jt * 128 : (jt + 1) * 128],
                                rhs=q_sb[0:64, t, :],
                                start=True,
                                stop=True,
                            )
                            nc.tensor.matmul(
                                dB[:, jj * 512 : (jj + 1) * 512],
                                lhsT=k_sb[64:128, t, jt * 128 : (jt + 1) * 128],
                                rhs=q_sb[64:128, t, :],
                                start=True,
                                stop=True,
                            )
                        nc.scalar.activation(
                            expA[:, 2 * jh : 2 * jh + 2, :],
                            dA.rearrange("p (a n) -> p a n", a=2),
                            EXP_F,
                            scale=SCALE,
                        )
                        nc.scalar.activation(
                            expB[:, 2 * jh : 2 * jh + 2, :],
                            dB.rearrange("p (a n) -> p a n", a=2),
                            EXP_F,
                            scale=SCALE,
                        )

                    def attnv(jh):
                        for jj in range(2):
                            jt = 2 * jh + jj
                            nc.tensor.matmul(
                                oAB[0:65, 0:512],
                                lhsT=vaug[:, jt, 2 * t, :],
                                rhs=expA[:, jt, :],
                                start=(jt == 0),
                                stop=(jt == NT - 1),
                            )
                            nc.tensor.matmul(
                                oAB[0:65, 512:1024],
                                lhsT=vaug[:, jt, 2 * t + 1, :],
                                rhs=expB[:, jt, :],
                                start=(jt == 0),
                                stop=(jt == NT - 1),
                            )

                    dots(0)
                    filler(1)
                    dots(1)
                    filler(2)
                    attnv(0)
                    if t > 0:
                        norm_pe(s, t - 1)
                    filler(1)
                    attnv(1)

                    # evacuate unnormalized oT (f32 PSUM -> bf16 SBUF)
                    nc.vector.tensor_copy(oT[0:64, t, :], oAB[0:64, 0:512])
                    nc.vector.tensor_copy(oT[64:128, t, :], oAB[0:64, 512:1024])
                    norm_collect(s, t, oAB)

                    if s == SEQ_PER_CORE - 1 and t == 3:
                        # Final projection, split: e-tiles 0..2 accumulate
                        # into the PSUM banks the last exps just freed,
                        # running under pair 3's norm chain; only the et=3
                        # matmuls + bias + store remain after the last
                        # oT *= R.
                        pj01 = psd.tile([128, 1024], F32, tag="dA", name="pj01")
                        pj23 = psd.tile([128, 1024], F32, tag="dB", name="pj23")
                        for et in range(DT - 1):
                            for dt in range(DT):
                                pj = pj01 if dt < 2 else pj23
                                col = (dt % 2) * 512
                                nc.tensor.matmul(
                                    pj[:, col : col + 512],
                                    lhsT=wo_sb[:, et, dt * 128 : (dt + 1) * 128],
                                    rhs=oT[:, et, :],
                                    start=(et == 0),
                                    stop=False,
                                )

                # drain leftover fillers, then close out this sequence
                filler(len(fill))
                if s == SEQ_PER_CORE - 1:
                    norm_pe(s, 3)
                    for dt in range(DT):
                        pj = pj01 if dt < 2 else pj23
                        col = (dt % 2) * 512
                        nc.tensor.matmul(
                            pj[:, col : col + 512],
                            lhsT=wo_sb[:, DT - 1, dt * 128 : (dt + 1) * 128],
                            rhs=oT[:, DT - 1, :],
                            start=False,
                            stop=True,
                        )
                    for dt in range(DT):
                        pj = pj01 if dt < 2 else pj23
                        col = (dt % 2) * 512
                        yt = ypool.tile([128, 512], F32, tag="y", name=f"ytL{dt}")
                        eng = nc.vector if dt % 2 == 0 else nc.scalar
                        if eng is nc.vector:
                            eng.tensor_scalar_add(
                                yt[:], pj[:, col : col + 512], b_sb[:, dt : dt + 1]
                            )
                        else:
                            eng.activation(
                                yt[:],
                                pj[:, col : col + 512],
                                IDENT_F,
                                bias=b_sb[:, dt : dt + 1],
                            )
                        nc.sync.dma_start(out[s, dt], yt[:])

    nc.compile()
    return nc


def make_in_maps(x, W_qkv, W_out, b_out):
    """Shard + lay out full inputs into the 8 per-core input maps."""
    import ml_dtypes

    b, p, n, d = x.shape
    xs = np.ascontiguousarray(x, dtype=np.float32).reshape(b * p, n, d)
    Wb = (
        np.ascontiguousarray(W_qkv, dtype=np.float32)
        .reshape(DT, 128, 3, 4, 128)
        .astype(ml_dtypes.bfloat16)
    )
    # [8, 128, DT, 128] (partition-major): q e-tiles 0..3 then k e-tiles 0..3
    wqk = np.ascontiguousarray(
        np.concatenate(
            [Wb[:, :, 0].transpose(2, 1, 0, 3), Wb[:, :, 1].transpose(2, 1, 0, 3)]
        )
    )
    # [128, DT, D] (partition-major)
    wv = np.ascontiguousarray(Wb[:, :, 2].reshape(DT, 128, D).transpose(1, 0, 2))
    wout = np.ascontiguousarray(
        np.ascontiguousarray(W_out, dtype=np.float32)
        .reshape(DT, 128, D)
        .astype(ml_dtypes.bfloat16)
        .transpose(1, 0, 2)
    )
    bo = np.ascontiguousarray(b_out, dtype=np.float32)

    in_maps = []
    for c in range(N_CORES):
        seqs = xs[c * SEQ_PER_CORE : (c + 1) * SEQ_PER_CORE]  # (4, n, d)
        xT = (
            np.ascontiguousarray(seqs.transpose(0, 2, 1))
            .reshape(SEQ_PER_CORE, DT, 128, N)
            .astype(ml_dtypes.bfloat16)
        )
        p2 = np.zeros((1, 256), dtype=ml_dtypes.bfloat16)
        p2[0, 0:64] = 1.0
        p2[0, 128 + 64 : 256] = 1.0
        in_maps.append(
            {"xT": xT, "wqk": wqk, "wv": wv, "wout": wout, "bout": bo, "p2d": p2}
        )
    return in_maps


def assemble_output(results, b, p, n, d):
    """Gather per-core yT outputs back into the full (b,p,n,d) array."""
    y = np.empty((b * p, n, d), dtype=np.float32)
    for c in range(N_CORES):
        yT = np.asarray(results[c]["out"]).reshape(SEQ_PER_CORE, D, N)
        y[c * SEQ_PER_CORE : (c + 1) * SEQ_PER_CORE] = yT.transpose(0, 2, 1)
    return y.reshape(b, p, n, d)


_NC_CACHE = None


def _get_nc():
    global _NC_CACHE
    if _NC_CACHE is None:
        _NC_CACHE = build_nc()
    return _NC_CACHE


def run(inputs, trace=False, **spmd_kwargs):
    """Run on the 8 NeuronCores; returns (full_output, BassKernelResults)."""
    from concourse.bass_utils import run_bass_kernel_spmd

    x = np.asarray(inputs["x"])
    b, p, n, d = x.shape
    nc = _get_nc()
    in_maps = make_in_maps(x, inputs["W_qkv"], inputs["W_out"], inputs["b_out"])
    res = run_bass_kernel_spmd(
        nc, in_maps, core_ids=list(range(N_CORES)), trace=trace, **spmd_kwargs
    )
    return assemble_output(res.results, b, p, n, d), res


def kernel(x, W_qkv, W_out, b_out):
    out, _ = run({"x": x, "W_qkv": W_qkv, "W_out": W_out, "b_out": b_out})
    return out.astype(np.float32)


# revision 31
# speedup vs baseline: 1.3471x; 1.0049x over previous
"""Multi-head attention (b=2, p=16, n=512, d=512, h=8, dh=64) on 8 TRN2 cores.

Data-parallel over the 32 (b,p) sequences: 4 sequences per core, no
collectives.  Per-core dataflow (everything "T" = feature-on-partition):

  xT  (d,n)  --W_qkv stationary-->  qT,kT (e,n)   [e-tile = 2 heads]
  xT chunks stationary, W_v moving ->  v natural (n,e)  -> vaug (j,h,65)
  dotsT[j,i] = kT_h.T-slice @ qT_h   (K=64, heads A/B at rows 0:64/64:128)
  expT = exp(scale * dotsT)          (ScalarE, PSUM->SBUF, bf16 out)
  oT[dh,i] (+ sums row 64) = vaug_h.T @ expT_h   (M=65, ones column -> sums)
  softmax denom (per head pair, zero DMA):
    sums row [1,1024] -> SBUF (one ScalarE copy) -> reciprocal_approx_fast
    in place (DVE) -> bf16 cast (DVE) -> two accumulating K=1 mask matmuls
    broadcast A/B recips across partitions -> oT *= R (DVE, in place)
  yT = W_out.T @ oT, bias added via ScalarE Identity-activation, each
  128-col chunk DMAed out as soon as it is done.

Scheduling: one software pipeline.  QKV e-tiles are computed just-in-time
(a sequence's q0/k0/v tiles complete during the previous sequence, its
late q/k e-tiles ride its own early pairs), the previous sequence's
output projection and the next sequence's QKV fill the attention pairs so
the TensorEngine never starves.  Input DMAs are chunked so the first
matmul starts right after the framework preamble.  The final projection
accumulates e-tiles 0..2 into PSUM banks freed by the last exps while the
last softmax-denominator chain runs, leaving only 4 matmuls + bias +
store after the final normalization.
"""

import os
import sys

import numpy as np

for _p in ("/opt/trn_rl_repo", "/root/.axon_site/_ro/trn_rl_repo"):
    if os.path.isdir(_p) and _p not in sys.path:
        sys.path.insert(0, _p)

import concourse.bass as bass  # noqa: E402
import concourse.mybir as mybir  # noqa: E402
from concourse import bacc  # noqa: E402
from concourse.tile import TileContext  # noqa: E402

F32 = mybir.dt.float32
BF16 = mybir.dt.bfloat16
F32R = mybir.dt.float32r

N_CORES = 8
SEQ_PER_CORE = 4  # (b*p)=32 sequences / 8 cores
N = 512  # tokens per sequence
D = 512  # model dim
HEADS = 8
DH = 64
SCALE = DH**-0.5
NT = N // 128  # 4 token tiles
DT = D // 128  # 4 dim tiles

EXP_F = mybir.ActivationFunctionType.Exp
COPY_F = mybir.ActivationFunctionType.Copy
IDENT_F = mybir.ActivationFunctionType.Identity
MULT = mybir.AluOpType.mult


def build_nc():
    """Build the per-core SPMD Bass program (same program on all 8 cores)."""
    nc = bacc.Bacc("TRN2", target_bir_lowering=False)

    xT = nc.declare_dram_parameter(
        "xT", [SEQ_PER_CORE, DT, 128, N], BF16, isOutput=False
    )
    # q e-tiles 0..3 then k e-tiles 0..3, each prepacked [128, DT, 128]
    wqk = nc.declare_dram_parameter("wqk", [8, 128, DT, 128], BF16, isOutput=False)
    wv = nc.declare_dram_parameter("wv", [128, DT, D], BF16, isOutput=False)
    wout = nc.declare_dram_parameter("wout", [128, DT, D], BF16, isOutput=False)
    bout = nc.declare_dram_parameter("bout", [D], F32, isOutput=False)
    p2d = nc.declare_dram_parameter("p2d", [1, 256], BF16, isOutput=False)
    out = nc.declare_dram_parameter(
        "out", [SEQ_PER_CORE, DT, 128, N], F32, isOutput=True
    )

    with TileContext(nc) as tc:
        with (
            tc.tile_pool(name="consts", bufs=1) as cpool,
            tc.tile_pool(name="xin", bufs=2) as xpool,
            tc.tile_pool(name="qk", bufs=2) as qkpool,
            tc.tile_pool(name="vaug", bufs=2) as vpool,
            tc.tile_pool(name="expt", bufs=4) as epool,
            tc.tile_pool(name="ot", bufs=2) as opool,
            tc.tile_pool(name="small", bufs=3) as spool,
            tc.tile_pool(name="yout", bufs=4) as ypool,
            tc.tile_pool(name="psq", bufs=2, space="PSUM") as psq,
            tc.tile_pool(name="psd", bufs=1, space="PSUM") as psd,
            tc.tile_pool(name="pso", bufs=1, space="PSUM") as pso,
        ):
            # ---- constants (DMA issue order matters: first-needed first;
            # all weight tensors host-prepacked so each DMA is contiguous) --
            wqk_sb = cpool.tile([128, 8, DT, 128], BF16, tag="wqk")

            seq_x = {}

            def x_alloc(s, interleave_et0=False):
                x2 = [
                    xpool.tile([128, 2, N], BF16, tag=f"x{h}", name=f"x{s}_{h}")
                    for h in range(2)
                ]
                for h in range(2):
                    # q0/k0 weight chunks interleaved with x so the first
                    # matmuls start as soon as their operands land
                    if interleave_et0:
                        nc.sync.dma_start(
                            wqk_sb[:, 0, 2 * h : 2 * h + 2],
                            wqk[0, :, 2 * h : 2 * h + 2],
                        )
                    nc.sync.dma_start(
                        x2[h][:], xT[s, 2 * h : 2 * h + 2].rearrange("t p n -> p t n")
                    )
                seq_x[s] = [x2[0][:, 0], x2[0][:, 1], x2[1][:, 0], x2[1][:, 1]]

            x_alloc(0, interleave_et0=True)
            nc.sync.dma_start(wqk_sb[:, 4], wqk[4])

            p2 = cpool.tile([1, 256], BF16, tag="p2")
            nc.sync.dma_start(p2[:], p2d[:])
            wv_sb = cpool.tile([128, DT, D], BF16, tag="wv")
            nc.sync.dma_start(wv_sb[:], wv[:])
            for et in (1, 5, 2, 6, 3, 7):
                nc.sync.dma_start(wqk_sb[:, et], wqk[et])
            wo_sb = cpool.tile([128, DT, D], BF16, tag="wo")
            nc.sync.dma_start(wo_sb[:], wout[:])
            b_sb = cpool.tile([128, DT], F32, tag="b")
            nc.sync.dma_start(b_sb[:], bout.rearrange("(t p) -> p t", p=128))

            seq_tiles = {}

            def qkv_alloc(s):
                q_sb = qkpool.tile([128, DT, N], BF16, tag="q", name=f"q{s}")
                k_sb = qkpool.tile([128, DT, N], BF16, tag="k", name=f"k{s}")
                vaug = vpool.tile(
                    [128, NT, HEADS, DH + 1], BF16, tag="v", name=f"v{s}"
                )
                nc.vector.memset(vaug[:, :, :, DH : DH + 1], 1.0)
                seq_tiles[s] = (q_sb, k_sb, vaug)

            def qkv_etile(s, kind, idx, pool_tag=None):
                """One QKV output tile: 4 accumulating matmuls + evacuation.
                kind 'q'/'k': e-tile idx (head pair idx); 'v': n-tile idx."""
                q_sb, k_sb, vaug = seq_tiles[s]
                xts = seq_x[s]
                if pool_tag is None:
                    ps = psq.tile(
                        [128, 512], F32, tag="ps", name=f"ps_{kind}{s}_{idx}"
                    )
                else:
                    ps = psd.tile(
                        [128, 512], F32, tag=pool_tag, name=f"ps_{kind}{s}_{idx}"
                    )
                if kind in ("q", "k"):
                    et = idx if kind == "q" else 4 + idx
                    for dt in range(DT):
                        nc.tensor.matmul(
                            ps[:],
                            lhsT=wqk_sb[:, et, dt, :],
                            rhs=xts[dt][:],
                            start=(dt == 0),
                            stop=(dt == DT - 1),
                        )
                    dest = q_sb if kind == "q" else k_sb
                    nc.vector.tensor_copy(dest[:, idx, :], ps[:])
                else:
                    nt = idx
                    for dt in range(DT):
                        nc.tensor.matmul(
                            ps[:],
                            lhsT=xts[dt][:, nt * 128 : (nt + 1) * 128],
                            rhs=wv_sb[:, dt, :],
                            start=(dt == 0),
                            stop=(dt == DT - 1),
                        )
                    nc.vector.tensor_copy(
                        vaug[:, nt, :, 0:DH],
                        ps.rearrange("p (h d) -> p h d", h=HEADS),
                    )

            rec2s = {}

            def norm_collect(s, t, oAB):
                """Sums rows -> 1/sums -> rec2 (bf16 [1,1024]), per pair.
                No DMA anywhere in the chain; the off-critical copies ride
                the idle Pool engine, except for the final pair whose chain
                is exposed at the kernel tail."""
                recf = spool.tile([1, 1024], F32, tag="recf", name=f"rf{s}_{t}")
                nc.scalar.activation(recf[0:1, :], oAB[64:65, :], COPY_F)
                nc.vector.reciprocal_approx_fast(recf[:], recf[:])
                rec2 = spool.tile([1, 1024], BF16, tag="rec2", name=f"r2_{s}_{t}")
                with nc.allow_low_precision(reason="softmax recip bf16"):
                    nc.vector.tensor_copy(rec2[:], recf[:])
                rec2s[(s, t)] = rec2

            def norm_pe(s, t):
                """R broadcast matmul + oT *= R for pair t of sequence s."""
                oT = seq_o[s]
                Rp = psq.tile([128, 512], F32, tag="ps", name=f"Rp{s}_{t}")
                rec2 = rec2s[(s, t)]
                nc.tensor.matmul(
                    Rp[:],
                    lhsT=p2[0:1, 0:128],
                    rhs=rec2[0:1, 0:512],
                    start=True,
                    stop=False,
                )
                nc.tensor.matmul(
                    Rp[:],
                    lhsT=p2[0:1, 128:256],
                    rhs=rec2[0:1, 512:1024],
                    start=False,
                    stop=True,
                )
                nc.vector.tensor_tensor(oT[:, t, :], oT[:, t, :], Rp[:], MULT)

            seq_o = {}

            def proj_dt(s, dt):
                """One 128-col chunk of the output projection + store."""
                oT = seq_o[s]
                ps = psq.tile([128, 512], F32, tag="ps", name=f"pj{s}_{dt}")
                for et in range(DT):
                    nc.tensor.matmul(
                        ps[:],
                        lhsT=wo_sb[:, et, dt * 128 : (dt + 1) * 128],
                        rhs=oT[:, et, :],
                        start=(et == 0),
                        stop=(et == DT - 1),
                    )
                yt = ypool.tile([128, 512], F32, tag="y", name=f"yt{s}_{dt}")
                nc.scalar.activation(yt[:], ps[:], IDENT_F, bias=b_sb[:, dt : dt + 1])
                nc.sync.dma_start(out[s, dt], yt[:])

            # ---- prologue: seq 0 q0/k0/v0/v1 -----------------------------
            qkv_alloc(0)
            qkv_etile(0, "q", 0, pool_tag="dA")
            qkv_etile(0, "k", 0, pool_tag="dB")
            qkv_etile(0, "v", 0)
            qkv_etile(0, "v", 1)

            for s in range(SEQ_PER_CORE):
                q_sb, k_sb, vaug = seq_tiles[s]
                oT = opool.tile([128, DT, N], BF16, tag="o", name=f"oT{s}")
                seq_o[s] = oT

                # filler units consumed inside the pair loop below.  Late
                # QKV e-tiles of sequence s ride in s's own stream (q2/k2
                # before pair 2, q3/k3 before pair 3); the early e-tiles of
                # s+1 (q0/k0 + all v) complete during s so s+1's pair 0 can
                # start immediately.
                fill = []

                def et_units(s_, pairs):
                    return [
                        (lambda k__=k, i__=i, s__=s_: qkv_etile(s__, k__, i__))
                        for (k, i) in pairs
                    ]

                if s == 0:
                    fill += et_units(0, [("v", 2), ("v", 3), ("q", 1), ("k", 1)])
                else:
                    fill += et_units(s, [("q", 2), ("k", 2)])
                    fill.append(lambda s_=s - 1: norm_pe(s_, 3))
                    fill += [
                        (lambda s_=s - 1, d_=d: proj_dt(s_, d_)) for d in range(DT)
                    ]
                if s == 0:
                    fill += et_units(0, [("q", 2), ("k", 2), ("q", 3), ("k", 3)])
                else:
                    fill += et_units(s, [("q", 3), ("k", 3)])
                if s + 1 < SEQ_PER_CORE:
                    qkv_alloc(s + 1)
                    fill += et_units(
                        s + 1,
                        [("q", 0), ("k", 0),
                         ("v", 0), ("v", 1), ("v", 2), ("v", 3),
                         ("q", 1), ("k", 1)],
                    )
                fi = 0

                def filler(k):
                    nonlocal fi
                    for _ in range(k):
                        if fi < len(fill):
                            fill[fi]()
                            fi += 1

                for t in range(4):  # head pair (2t, 2t+1)
                    if t == 1 and s + 1 < SEQ_PER_CORE:
                        x_alloc(s + 1)
                    expA = epool.tile([128, NT, N], BF16, tag="expA")
                    expB = epool.tile([128, NT, N], BF16, tag="expB")
                    oAB = pso.tile([128, 1024], F32, tag="oAB")

                    def dots(jh):
                        dA = psd.tile([128, 1024], F32, tag="dA")
                        dB = psd.tile([128, 1024], F32, tag="dB")
                        for jj in range(2):
                            jt = 2 * jh + jj
                            nc.tensor.matmul(
                                dA[:, jj * 512 : (jj + 1) * 512],
                                lhsT=k_sb[0:64, t, jt * 128 : (jt + 1) * 128],
                                rhs=q_sb[0:64, t, :],
                                start=True,
                                stop=True,
                            )
                            nc.tensor.matmul(
                                dB[:, jj * 512 : (jj + 1) * 512],
                                lhsT=k_sb[64:128, t, jt * 128 : (jt + 1) * 128],
                                rhs=q_sb[64:128, t, :],
                                start=True,
                                stop=True,
                            )
                        nc.scalar.activation(
                            expA[:, 2 * jh : 2 * jh + 2, :],
                            dA.rearrange("p (a n) -> p a n", a=2),
                            EXP_F,
                            scale=SCALE,
                        )
                        nc.scalar.activation(
                            expB[:, 2 * jh : 2 * jh + 2, :],
                            dB.rearrange("p (a n) -> p a n", a=2),
                            EXP_F,
                            scale=SCALE,
                        )

                    def attnv(jh):
                        for jj in range(2):
                            jt = 2 * jh + jj
                            nc.tensor.matmul(
                                oAB[0:65, 0:512],
                                lhsT=vaug[:, jt, 2 * t, :],
                                rhs=expA[:, jt, :],
                                start=(jt == 0),
                                stop=(jt == NT - 1),
                            )
                            nc.tensor.matmul(
                                oAB[0:65, 512:1024],
                                lhsT=vaug[:, jt, 2 * t + 1, :],
                                rhs=expB[:, jt, :],
                                start=(jt == 0),
                                stop=(jt == NT - 1),
                            )

                    dots(0)
                    filler(1)
                    dots(1)
                    filler(2)
                    attnv(0)
                    if t > 0:
                        norm_pe(s, t - 1)
                    filler(1)
                    attnv(1)

                    # evacuate unnormalized oT (f32 PSUM -> bf16 SBUF)
                    nc.vector.tensor_copy(oT[0:64, t, :], oAB[0:64, 0:512])
                    nc.vector.tensor_copy(oT[64:128, t, :], oAB[0:64, 512:1024])
                    norm_collect(s, t, oAB)

                    if s == SEQ_PER_CORE - 1 and t == 3:
                        # Final projection, split: e-tiles 0..2 accumulate
                        # into the PSUM banks the last exps just freed,
                        # running under pair 3's norm chain; only the et=3
                        # matmuls + bias + store remain after the last
                        # oT *= R.
                        pj01 = psd.tile([128, 1024], F32, tag="dA", name="pj01")
                        pj23 = psd.tile([128, 1024], F32, tag="dB", name="pj23")
                        for et in range(DT - 1):
                            for dt in range(DT):
                                pj = pj01 if dt < 2 else pj23
                                col = (dt % 2) * 512
                                nc.tensor.matmul(
                                    pj[:, col : col + 512],
                                    lhsT=wo_sb[:, et, dt * 128 : (dt + 1) * 128],
                                    rhs=oT[:, et, :],
                                    start=(et == 0),
                                    stop=False,
                                )

                # drain leftover fillers, then close out this sequence
                filler(len(fill))
                if s == SEQ_PER_CORE - 1:
                    norm_pe(s, 3)
                    for dt in range(DT):
                        pj = pj01 if dt < 2 else pj23
                        col = (dt % 2) * 512
                        nc.tensor.matmul(
                            pj[:, col : col + 512],
                            lhsT=wo_sb[:, DT - 1, dt * 128 : (dt + 1) * 128],
                            rhs=oT[:, DT - 1, :],
                            start=False,
                            stop=True,
                        )
                    for dt in range(DT):
                        pj = pj01 if dt < 2 else pj23
                        col = (dt % 2) * 512
                        yt = ypool.tile([128, 512], F32, tag="y", name=f"ytL{dt}")
                        eng = nc.vector if dt % 2 == 0 else nc.scalar
                        if eng is nc.vector:
                            eng.tensor_scalar_add(
                                yt[:], pj[:, col : col + 512], b_sb[:, dt : dt + 1]
                            )
                        else:
                            eng.activation(
                                yt[:],
                                pj[:, col : col + 512],
                                IDENT_F,
                                bias=b_sb[:, dt : dt + 1],
                            )
                        nc.sync.dma_start(out[s, dt], yt[:])

    nc.compile()
    return nc


def make_in_maps(x, W_qkv, W_out, b_out):
    """Shard + lay out full inputs into the 8 per-core input maps."""
    import ml_dtypes

    b, p, n, d = x.shape
    xs = np.ascontiguousarray(x, dtype=np.float32).reshape(b * p, n, d)
    Wb = (
        np.ascontiguousarray(W_qkv, dtype=np.float32)
        .reshape(DT, 128, 3, 4, 128)
        .astype(ml_dtypes.bfloat16)
    )
    # [8, 128, DT, 128] (partition-major): q e-tiles 0..3 then k e-tiles 0..3
    wqk = np.ascontiguousarray(
        np.concatenate(
            [Wb[:, :, 0].transpose(2, 1, 0, 3), Wb[:, :, 1].transpose(2, 1, 0, 3)]
        )
    )
    # [128, DT, D] (partition-major)
    wv = np.ascontiguousarray(Wb[:, :, 2].reshape(DT, 128, D).transpose(1, 0, 2))
    wout = np.ascontiguousarray(
        np.ascontiguousarray(W_out, dtype=np.float32)
        .reshape(DT, 128, D)
        .astype(ml_dtypes.bfloat16)
        .transpose(1, 0, 2)
    )
    bo = np.ascontiguousarray(b_out, dtype=np.float32)

    in_maps = []
    for c in range(N_CORES):
        seqs = xs[c * SEQ_PER_CORE : (c + 1) * SEQ_PER_CORE]  # (4, n, d)
        xT = (
            np.ascontiguousarray(seqs.transpose(0, 2, 1))
            .reshape(SEQ_PER_CORE, DT, 128, N)
            .astype(ml_dtypes.bfloat16)
        )
        p2 = np.zeros((1, 256), dtype=ml_dtypes.bfloat16)
        p2[0, 0:64] = 1.0
        p2[0, 128 + 64 : 256] = 1.0
        in_maps.append(
            {"xT": xT, "wqk": wqk, "wv": wv, "wout": wout, "bout": bo, "p2d": p2}
        )
    return in_maps


def assemble_output(results, b, p, n, d):
    """Gather per-core yT outputs back into the full (b,p,n,d) array."""
    y = np.empty((b * p, n, d), dtype=np.float32)
    for c in range(N_CORES):
        yT = np.asarray(results[c]["out"]).reshape(SEQ_PER_CORE, D, N)
        y[c * SEQ_PER_CORE : (c + 1) * SEQ_PER_CORE] = yT.transpose(0, 2, 1)
    return y.reshape(b, p, n, d)


_NC_CACHE = None


def _get_nc():
    global _NC_CACHE
    if _NC_CACHE is None:
        _NC_CACHE = build_nc()
    return _NC_CACHE


def run(inputs, trace=False, **spmd_kwargs):
    """Run on the 8 NeuronCores; returns (full_output, BassKernelResults)."""
    from concourse.bass_utils import run_bass_kernel_spmd

    x = np.asarray(inputs["x"])
    b, p, n, d = x.shape
    nc = _get_nc()
    in_maps = make_in_maps(x, inputs["W_qkv"], inputs["W_out"], inputs["b_out"])
    res = run_bass_kernel_spmd(
        nc, in_maps, core_ids=list(range(N_CORES)), trace=trace, **spmd_kwargs
    )
    return assemble_output(res.results, b, p, n, d), res


def kernel(x, W_qkv, W_out, b_out):
    out, _ = run({"x": x, "W_qkv": W_qkv, "W_out": W_out, "b_out": b_out})
    return out.astype(np.float32)
